# revision 1
# baseline (speedup 1.0000x reference)
"""BiRNN (bidirectional GRU) language model kernel for Trainium2, 8 NeuronCores.

Sharding: data-parallel over batch. Each of the 8 cores takes 2 of the 16 batch
columns and computes everything for its 512 tokens (embedding gather, both GRU
scans, vocab projection, log-softmax) with zero collectives.

Token order per core: t = 2*s + b (s = seq position 0..255, b = local batch 0..1).

Device layout highlights:
  - xT_ext [65, 512]: embedded tokens transposed (E on partitions) + ones row,
    so the gx matmul folds in b_ih.
  - gx precomputed for both directions; r/z part stored in ``gxpre`` (with a
    zero block for the n rows) and PSUM-preloaded before each step's gh matmul
    (start=False accumulate), so the r/z gate adds come free.  xn kept apart.
  - whh_ext [33, 192]: W_hh plus a bias row; h state tiles carry a ones row, so
    the gh matmul folds in b_hh.
  - h' = (1-z)*n + z*h with 1-z computed as sigmoid(-x) on the ACT engine and
    z*h_prev computed while the n-path is still going (both off the critical
    path).
  - h stored into 4 contiguous "shell" tiles [65, 128] (h_l rows 0:32, h_r rows
    32:64, ones row 64); shell k covers seq positions [64k, 64k+64) i.e. token
    rows [128k, 128k+128), so each projection store is one dense 128-partition
    DMA.  Shells are the stationary (lhsT) operand of the vocab projection,
    wout_ext [65, V] carries rnn_out + bias row.
  - log-softmax without a max pass: |logits| <= 65 so exp() cannot overflow
    f32.  Pass 1 computes sum(exp(logits)) per token via Exp+accum_out; pass 2
    recomputes logits and writes logits - log(sum) straight out.
  - wout columns [0, NCACHE) are cached in SBUF (loaded during the scan); the
    remaining columns stream twice (once per pass).
"""

import os
import sys
from contextlib import ExitStack

import numpy as np

for _p in (
    "/opt/trn_rl_repo",
    "/root/.axon_site",
    "/root/.axon_site/_ro/trn_rl_repo",
    "/root/.axon_site/_ro/pypackages",
):
    if os.path.isdir(_p) and _p not in sys.path:
        sys.path.append(_p)

import concourse.bass as bass
import concourse.bacc as bacc
import concourse.tile as tile
from concourse import mybir
from concourse.masks import make_identity

F32 = mybir.dt.float32
BF16 = mybir.dt.bfloat16
I32 = mybir.dt.int32
AF = mybir.ActivationFunctionType
ALU = mybir.AluOpType

V = 50257
E = 64
H = 32
S = 256
B = 16
NCORES = 8
BC = B // NCORES          # batch columns per core
T = S * BC                # tokens per core
G3 = 3 * H                # 96 gate rows
KP = 2 * H + 1            # 65: [h_l; h_r; ones] contraction size for projection
VGRP = 2048               # vocab columns per projection group
NCACHE = int(os.environ.get("KNCACHE", "24576"))  # wout columns cached in SBUF
NGRP_C = NCACHE // VGRP   # cached groups
NGRP_S = (V - NCACHE + VGRP - 1) // VGRP  # streamed groups
NGRP = NGRP_C + NGRP_S


def shell_of(s):
    """Seq position s -> (shell index, column offset).  Shell k holds
    s in [64k, 64k+64), i.e. token rows [128k, 128k+128) of the output."""
    return s // 64, 2 * (s % 64)


def build_module(phases=("pre", "scan", "proj"), use_preload=True):
    # phases may also contain "pass1only" to skip the second projection pass
    nc = bacc.Bacc("TRN2", target_bir_lowering=False)
    tok_h = nc.dram_tensor("tok", (T,), I32, kind="ExternalInput")
    emb_h = nc.dram_tensor("embed", (V, E), F32, kind="ExternalInput")
    wih_h = nc.dram_tensor("wih", (E + 1, 2 * G3), F32, kind="ExternalInput")
    whh_h = nc.dram_tensor("whh", (H + 1, 2 * G3), F32, kind="ExternalInput")
    wout1_h = nc.dram_tensor("wout1", (KP, V), BF16, kind="ExternalInput")
    wout2_h = nc.dram_tensor("wout2", (KP, V), BF16, kind="ExternalInput")
    out_h = nc.dram_tensor("out", (T, V), F32, kind="ExternalOutput")

    with tile.TileContext(nc) as tc:
        with ExitStack() as ctx:
            const = ctx.enter_context(tc.tile_pool(name="const", bufs=1))
            hall = ctx.enter_context(tc.tile_pool(name="hall", bufs=1))

            ident = const.tile([128, 128], F32, tag="ident")
            make_identity(nc, ident[:])
            wih_sb = const.tile([E + 1, 2 * G3], F32, tag="wih")
            nc.sync.dma_start(out=wih_sb[:], in_=wih_h[:])
            whh_sb = const.tile([H + 1, 2 * G3], F32, tag="whh")
            nc.sync.dma_start(out=whh_sb[:], in_=whh_h[:])
            tok_sb = const.tile([128, 4], I32, tag="tok")
            nc.sync.dma_start(out=tok_sb[:], in_=tok_h[:].rearrange("(g p) -> p g", p=128))

            xt = const.tile([E + 1, T], F32, tag="xt")
            nc.vector.memset(xt[E : E + 1, :], 1.0)

            # wout cache for columns [0, NCACHE); DMA issued up front so it
            # overlaps the scan.
            wc1 = hall.tile([KP, NCACHE], BF16, tag="wc1")
            wc2 = hall.tile([KP, NCACHE], BF16, tag="wc2")
            for wc, wh in ((wc1, wout1_h), (wc2, wout2_h)):
                for c0 in range(0, NCACHE, 8192):
                    nc.sync.dma_start(
                        out=wc[:, c0 : c0 + 8192], in_=wh[:][:, c0 : c0 + 8192]
                    )

            hsh = []
            for k in range(4):
                hs = hall.tile([KP, 128], F32, tag=f"hs{k}", name=f"hs{k}")
                nc.vector.memset(hs[2 * H : 2 * H + 1, :], 1.0)
                hsh.append(hs)

            # ping-pong compact GRU state [h; ones] x (L b0, L b1, R b0, R b1)
            hA = const.tile([H + 1, 4], F32, tag="hA")
            hB = const.tile([H + 1, 4], F32, tag="hB")
            nc.vector.memset(hA[:], 0.0)
            nc.vector.memset(hA[H : H + 1, :], 1.0)
            nc.vector.memset(hB[H : H + 1, :], 1.0)

            with (
                tc.tile_pool(name="gath", bufs=2) as gpool,
                tc.tile_pool(name="gx", bufs=1) as gxpool,
                tc.tile_pool(name="scan", bufs=int(os.environ.get("KSCBUF", "3")) ) as scanp,
                tc.tile_pool(name="ps", bufs=2, space="PSUM") as pspool,
                tc.tile_pool(name="ghp", bufs=int(os.environ.get("KGHBUF", "3")), space="PSUM") as ghpool,
            ):
                # ---- embedding gather + transpose to [E, tokens] ----
                for g in range(4):
                    xg = gpool.tile([128, E], F32, tag="xg")
                    nc.gpsimd.indirect_dma_start(
                        out=xg[:],
                        out_offset=None,
                        in_=emb_h[:],
                        in_offset=bass.IndirectOffsetOnAxis(ap=tok_sb[:, g : g + 1], axis=0),
                    )
                    xps = pspool.tile([E, 128], F32, tag="ps")
                    nc.tensor.transpose(xps[:], xg[:], ident[:])
                    nc.scalar.copy(out=xt[0:E, g * 128 : (g + 1) * 128], in_=xps[:])

                # ---- gx precompute for both directions ----
                # gxpre rows 0:64 = r/z-gate gx (PSUM preload); rows 64:96 zero.
                # xn_all = n-gate gx, added after r*hn.
                # Direction R is stored time-reversed so step t reads column t.
                gxpre = gxpool.tile([G3, S, 4], F32, tag="gxpre")
                xn_all = gxpool.tile([H, S, 4], F32, tag="xnall")
                nc.vector.memset(gxpre[2 * H : G3, :, :], 0.0)
                for d in range(2):
                    gps = pspool.tile([G3, T], F32, tag="ps")
                    nc.tensor.matmul(
                        gps[:], wih_sb[:, d * G3 : (d + 1) * G3], xt[:], start=True, stop=True
                    )
                    if d == 0:
                        src_rz = gps[0 : 2 * H, :].rearrange("p (s b) -> p s b", b=2)
                        src_n = gps[2 * H : G3, :].rearrange("p (s b) -> p s b", b=2)
                    else:
                        base_rz = gps[0 : 2 * H, :]
                        src_rz = bass.AP(
                            tensor=base_rz.tensor,
                            offset=base_rz.offset + (T - 2),
                            ap=[list(base_rz.ap[0]), [-2, S], [1, 2]],
                        )
                        base_n = gps[2 * H : G3, :]
                        src_n = bass.AP(
                            tensor=base_n.tensor,
                            offset=base_n.offset + (T - 2),
                            ap=[list(base_n.ap[0]), [-2, S], [1, 2]],
                        )
                    nc.vector.tensor_copy(out=gxpre[0 : 2 * H, :, 2 * d : 2 * d + 2], in_=src_rz)
                    nc.vector.tensor_copy(out=xn_all[:, :, 2 * d : 2 * d + 2], in_=src_n)

                # ---- the two GRU scans, fused: L at step t, R at step 255-t ----
                for t in range(S if "scan" in phases else 0):
                    sL = t
                    sR = S - 1 - t
                    hp = hA if t % 2 == 0 else hB
                    hn = hB if t % 2 == 0 else hA
                    gh = ghpool.tile([G3, 4], F32, tag="gh")
                    if use_preload:
                        nc.vector.tensor_copy(out=gh[:], in_=gxpre[:, t, :])
                    nc.tensor.matmul(
                        gh[:, 0:2], whh_sb[:, 0:G3], hp[:, 0:2],
                        start=not use_preload, stop=True, skip_group_check=True,
                    )
                    nc.tensor.matmul(
                        gh[:, 2:4], whh_sb[:, G3 : 2 * G3], hp[:, 2:4],
                        start=not use_preload, stop=True, skip_group_check=True,
                    )
                    # Gates via tanh only (sigmoid(x) = .5 + .5*tanh(x/2)):
                    # keeps the ACT table compatible with projection Exp so
                    # pass 1 can overlap the scan tail.
                    rz = scanp.tile([2 * H, 4], F32, tag="rz")
                    nc.scalar.activation(
                        out=rz[:], in_=gh[0 : 2 * H, :], func=AF.Tanh, scale=0.5
                    )
                    # (1-z) = .5 - .5*tz, on Pool, off the critical n path
                    cz = scanp.tile([H, 4], F32, tag="cz")
                    nc.gpsimd.tensor_scalar(cz[:], rz[H : 2 * H, :], -0.5, 0.5,
                                            ALU.mult, ALU.add)
                    # d = h - (1-z)*h, computed while the n path runs so the
                    # post-tanh tail is only two ops: h' = d + (1-z)*n
                    dd = scanp.tile([H, 4], F32, tag="dd")
                    nc.vector.tensor_mul(dd[:], cz[:], hp[0:H, :])
                    nc.vector.tensor_sub(dd[:], hp[0:H, :], dd[:])
                    # n path: r*hn = .5*(tr+1)*hn, via two fused ops
                    nn = scanp.tile([H, 4], F32, tag="nn")
                    nc.vector.scalar_tensor_tensor(
                        out=nn[:], in0=rz[0:H, :], scalar=1.0, in1=gh[2 * H : G3, :],
                        op0=ALU.add, op1=ALU.mult,
                    )
                    nc.vector.scalar_tensor_tensor(
                        out=nn[:], in0=nn[:], scalar=0.5, in1=xn_all[:, t, :],
                        op0=ALU.mult, op1=ALU.add,
                    )
                    nc.scalar.activation(out=nn[:], in_=nn[:], func=AF.Tanh)
                    nc.vector.tensor_mul(nn[:], nn[:], cz[:])
                    nc.vector.tensor_add(hn[0:H, :], nn[:], dd[:])
                    kL, cL = shell_of(sL)
                    kR, cR = shell_of(sR)
                    nc.gpsimd.tensor_copy(out=hsh[kL][0:H, cL : cL + 2], in_=hn[0:H, 0:2])
                    nc.gpsimd.tensor_copy(
                        out=hsh[kR][H : 2 * H, cR : cR + 2], in_=hn[0:H, 2:4]
                    )

            do_proj = "proj" in phases
            if not do_proj and "scan" not in phases:
                for k in range(4):
                    nc.vector.memset(hsh[k][0 : 2 * H, :], 0.0)

            # Split shells into bf16 hi/lo pairs: logits are computed as
            # h1@W1 + h1@W2 + h2@W1 (bf16 matmuls run 4x faster than f32;
            # the dropped h2@W2 term is ~2^-18 relative).
            hs1, hs2 = [], []
            for k in range(4):
                a = hall.tile([KP, 128], BF16, tag=f"hs1_{k}", name=f"hs1_{k}")
                nc.vector.tensor_copy(out=a[:], in_=hsh[k][:])
                b = hall.tile([KP, 128], BF16, tag=f"hs2_{k}", name=f"hs2_{k}")
                nc.vector.tensor_sub(b[:], hsh[k][:], a[:])
                hs1.append(a)
                hs2.append(b)

            # Scheduler-only fence: keeps projection Exp activations from
            # being interleaved with scan Sigmoid/Tanh in the ACT stream
            # (each mix would reload the 1.3us activation table), while DMA
            # prefetches can still run during the scan.
            if do_proj and os.environ.get("KBAR", "1") == "1":
                tc.no_sync_barrier()

            # ---- vocab projection + log-softmax, two passes over wout ----
            with (
                tc.tile_pool(name="wout", bufs=int(os.environ.get("KWBUF", "4"))) as wpool,
                tc.tile_pool(name="outp", bufs=int(os.environ.get("KOBUF", "3"))) as opool,
                tc.tile_pool(name="pp", bufs=2, space="PSUM") as pppool,
            ):
                stats = [
                    const.tile([128, NGRP], F32, tag=f"st{k}", name=f"stats{k}")
                    for k in range(4)
                ]
                negc = [
                    const.tile([128, 1], F32, tag=f"ng{k}", name=f"negc{k}")
                    for k in range(4)
                ]

                def groups(tag):
                    """Yield (group idx, col start, width, (w1, w2) tiles, rhs col0)."""
                    for g in range(NGRP_C):
                        c0 = g * VGRP
                        yield g, c0, VGRP, (wc1, wc2), c0
                    for i in range(NGRP_S):
                        c0 = NCACHE + i * VGRP
                        gw = min(VGRP, V - c0)
                        g = NGRP_C + i
                        wt1 = wpool.tile([KP, VGRP], BF16, tag="wt1", name=f"wt1_{tag}{g}")
                        nc.sync.dma_start(out=wt1[:, 0:gw], in_=wout1_h[:][:, c0 : c0 + gw])
                        wt2 = wpool.tile([KP, VGRP], BF16, tag="wt2", name=f"wt2_{tag}{g}")
                        nc.sync.dma_start(out=wt2[:, 0:gw], in_=wout2_h[:][:, c0 : c0 + gw])
                        yield g, c0, gw, (wt1, wt2), 0

                def emit_pass(tag, finalize, skip_lo=False):
                    for g, c0, gw, (w1, w2), w0 in groups(tag):
                        for k in range(4):
                            ps = pppool.tile(
                                [128, VGRP], F32, tag="pp", name=f"pp_{tag}{g}_{k}"
                            )
                            for q0 in range(0, gw, 512):
                                qw = min(512, gw - q0)
                                sl = slice(w0 + q0, w0 + q0 + qw)
                                nc.tensor.matmul(
                                    ps[:, q0 : q0 + qw], hs1[k][:], w1[:, sl],
                                    start=True, stop=False,
                                )
                                nc.tensor.matmul(
                                    ps[:, q0 : q0 + qw], hs1[k][:], w2[:, sl],
                                    start=False, stop=skip_lo,
                                )
                                if not skip_lo:
                                    nc.tensor.matmul(
                                        ps[:, q0 : q0 + qw], hs2[k][:], w1[:, sl],
                                        start=False, stop=True,
                                    )
                            finalize(g, c0, gw, k, ps)

                def fin1(g, c0, gw, k, ps):
                    nc.scalar.activation(
                        out=ps[:, 0:gw], in_=ps[:, 0:gw], func=AF.Exp,
                        accum_out=stats[k][:, g : g + 1],
                    )

                if do_proj:
                    emit_pass("a", fin1,
                              skip_lo=os.environ.get("KP1LO", "0") == "1")

                for k in range(4 if do_proj else 0):
                    ssum = const.tile([128, 1], F32, tag=f"ss{k}", name=f"ssum{k}")
                    nc.vector.tensor_reduce(
                        out=ssum[:], in_=stats[k][:], axis=mybir.AxisListType.X, op=ALU.add
                    )
                    nc.scalar.activation(out=negc[k][:], in_=ssum[:], func=AF.Ln)
                    nc.vector.tensor_scalar_mul(negc[k][:], negc[k][:], -1.0)

                def fin2(g, c0, gw, k, ps):
                    ob = opool.tile([128, VGRP], F32, tag="ob", name=f"ob{g}_{k}")
                    nc.vector.tensor_scalar_add(ob[:, 0:gw], ps[:, 0:gw], negc[k][:, 0:1])
                    out_base = out_h[:]
                    dst = bass.AP(
                        tensor=out_base.tensor,
                        offset=(128 * k) * V + c0,
                        ap=[[V, 128], [1, gw]],
                    )
                    nc.sync.dma_start(out=dst, in_=ob[:, 0:gw])

                if do_proj and "pass1only" not in phases:
                    emit_pass("b", fin2)
    nc.compile()
    return nc


_CACHE = {}


def _get_module():
    if "nc" not in _CACHE:
        _CACHE["nc"] = build_module()
    return _CACHE["nc"]


def prep_inputs(inputs):
    """Host-side prep: build per-core input maps from the full input dict."""
    ib = np.asarray(inputs["input_batch"])
    embed = np.ascontiguousarray(np.asarray(inputs["embed"], dtype=np.float32))
    rnn_out = np.asarray(inputs["rnn_out"], dtype=np.float32)
    rnn_out_bias = np.asarray(inputs["rnn_out_bias"], dtype=np.float32)

    wih = np.zeros((E + 1, 2 * G3), np.float32)
    wih[:E, :G3] = np.asarray(inputs["Wl_ih"], dtype=np.float32)
    wih[E, :G3] = np.asarray(inputs["bl_ih"], dtype=np.float32)
    wih[:E, G3:] = np.asarray(inputs["Wr_ih"], dtype=np.float32)
    wih[E, G3:] = np.asarray(inputs["br_ih"], dtype=np.float32)

    whh = np.zeros((H + 1, 2 * G3), np.float32)
    whh[:H, :G3] = np.asarray(inputs["Wl_hh"], dtype=np.float32)
    whh[H, :G3] = np.asarray(inputs["bl_hh"], dtype=np.float32)
    whh[:H, G3:] = np.asarray(inputs["Wr_hh"], dtype=np.float32)
    whh[H, G3:] = np.asarray(inputs["br_hh"], dtype=np.float32)

    import ml_dtypes

    wout = np.zeros((KP, V), np.float32)
    wout[0 : 2 * H] = rnn_out
    wout[2 * H] = rnn_out_bias[0]
    wout1 = wout.astype(ml_dtypes.bfloat16)
    wout2 = (wout - wout1.astype(np.float32)).astype(ml_dtypes.bfloat16)

    in_maps = []
    for c in range(NCORES):
        tok = np.ascontiguousarray(
            ib[:, BC * c : BC * (c + 1)].astype(np.int32).reshape(T)
        )
        in_maps.append(
            {"tok": tok, "embed": embed, "wih": wih, "whh": whh,
             "wout1": wout1, "wout2": wout2}
        )
    return in_maps


def assemble_output(results):
    out = np.empty((S, B, V), np.float32)
    for c in range(NCORES):
        out[:, BC * c : BC * (c + 1), :] = results[c]["out"].reshape(S, BC, V)
    return out


def kernel(**inputs):
    from concourse.bass_utils import run_bass_kernel_spmd

    nc = _get_module()
    in_maps = prep_inputs(inputs)
    res = run_bass_kernel_spmd(nc, in_maps, core_ids=list(range(NCORES)))
    return assemble_output(res.results)



# revision 29
# speedup vs baseline: 2.4214x; 2.4214x over previous
"""BiRNN (bidirectional GRU) language model kernel for Trainium2, 8 NeuronCores.

Sharding: data-parallel over batch (2 of 16 batch columns per core), no
collectives.  Each core computes the embedding gather, both GRU scans, the
vocab projection and the log-softmax for its 512 tokens.

Key structure:
  - Chunked scan: each direction's 256-step recurrence is split into C chunks
    run in parallel as lanes of the same instructions, each with a W-step
    warmup ramp.  Positions before the sequence start are padded with a
    frozen-gate column (z-gate pre-activation +30 => z=1 => h stays exactly 0),
    so chunks whose warmup crosses position 0 are exact and later chunks
    converge through the GRU's state contraction (validated: W=32 gives
    ~5e-7 max h error).  Serial steps: 256 -> 256/C + W.
  - Per step, the gate matmul Whh @ h(t-1) is split into an accumulation of
    three PSUM matmuls: identity @ gx(t) (the precomputed input gates),
    0.5*Whh @ zh'(t-1) and Whh @ cn(t-1), where h = cn + 0.5*zh'
    (cn = (1-z)*n, zh' = 2z*h).  This keeps the recurrence's serial path at
    mm -> tanh(rz) -> (r+1)*hn -> +xn -> tanh -> *(1-z) -> mm.
  - h states land in a position-indexed buffer (later chunks' warmup writes
    are overwritten by the owning chunk's real writes, which come later in
    step order).
  - Vocab projection: h split bf16 hi/lo packed into one K=128 stationary
    tile per 128 tokens ([h-hi 64 | ones | h-lo 63]); wout packed to match
    ([w-hi 64 | bias | w-hi 63]) so one bf16 matmul per 512 columns gives
    logits to ~2e-3 abs.  wout ([128, V] bf16, 12.9MB) is fully cached in
    SBUF, loaded during the scan.
  - log-softmax without a max pass (|logits| <= 65 so exp can't overflow):
    pass 1 Exp+accum_out per 2048-col group; pass 2 recomputes logits and
    writes logits - log(sum) as bf16 (host converts to f32).  Pass 2 of
    shell k is interleaved with pass 1 of shell k+1; the finalize add is
    split across DVE and Pool so ACT only does the Exps.
"""

import os
import sys
from contextlib import ExitStack

import numpy as np

for _p in (
    "/opt/trn_rl_repo",
    "/root/.axon_site",
    "/root/.axon_site/_ro/trn_rl_repo",
    "/root/.axon_site/_ro/pypackages",
):
    if os.path.isdir(_p) and _p not in sys.path:
        sys.path.append(_p)

import concourse.bass as bass
import concourse.bacc as bacc
import concourse.tile as tile
from concourse import mybir
from concourse.masks import make_identity

F32 = mybir.dt.float32
BF16 = mybir.dt.bfloat16
I32 = mybir.dt.int32
AF = mybir.ActivationFunctionType
ALU = mybir.AluOpType

V = 50257
E = 64
H = 32
S = 256
B = 16
NCORES = 8
BC = B // NCORES          # batch columns per core
T = S * BC                # tokens per core
G3 = 3 * H                # 96 gate rows (r, z, n)

C = int(os.environ.get("KC", "16"))    # chunks per direction
W = int(os.environ.get("KW", "32"))    # warmup steps
P = S // C                              # positions per chunk
STEPS = P + W
LPD = 2 * C                             # lanes per direction (chunk x batchcol)
LAN = 2 * LPD                           # total lanes (L block, R block)
GXW = 2 * W + 2 * S                     # compact gx width per direction block
HBW = 2 * S + 4 * W                     # h buffer width (L pad left, R pad right)

VGRP = int(os.environ.get("KVGRP", "1024"))  # vocab columns per projection group
NPSB = int(os.environ.get("KNPSB", "4"))     # projection PSUM buffers
NGRP = (V + VGRP - 1) // VGRP


def ap3(base, offset, dims):
    """AP with the partition entry of ``base`` plus custom free dims."""
    return bass.AP(tensor=base.tensor, offset=base.offset + offset,
                   ap=[list(base.ap[0])] + [list(d) for d in dims])


def build_module(phases=("pre", "scan", "proj"),
                 fin_split=tuple(os.environ.get("KFIN", "v"))):
    nc = bacc.Bacc("TRN2", target_bir_lowering=False)
    tok_h = nc.dram_tensor("tok", (T,), I32, kind="ExternalInput")
    emb_h = nc.dram_tensor("embed", (V, E), F32, kind="ExternalInput")
    wih_h = nc.dram_tensor("wih", (E + 1, 2 * G3), F32, kind="ExternalInput")
    whhdd_h = nc.dram_tensor("whhdd", (H + 1, 2 * G3), F32, kind="ExternalInput")
    whhcn_h = nc.dram_tensor("whhcn", (H, 2 * G3), F32, kind="ExternalInput")
    wout_h = nc.dram_tensor("wout", (128, V), BF16, kind="ExternalInput")
    ones_h = nc.dram_tensor("onesrow", (1, 128), BF16, kind="ExternalInput")
    out_h = nc.dram_tensor("out", (T, V), BF16, kind="ExternalOutput")

    with tile.TileContext(nc) as tc:
        with ExitStack() as ctx:
            const = ctx.enter_context(tc.tile_pool(name="const", bufs=1))
            hall = ctx.enter_context(tc.tile_pool(name="hall", bufs=1))

            ident = const.tile([128, 128], F32, tag="ident")
            make_identity(nc, ident[:])
            wih_sb = const.tile([E + 1, 2 * G3], F32, tag="wih")
            nc.sync.dma_start(out=wih_sb[:], in_=wih_h[:])
            whhdd_sb = const.tile([H + 1, 2 * G3], F32, tag="whhdd")
            nc.sync.dma_start(out=whhdd_sb[:], in_=whhdd_h[:])
            whhcn_sb = const.tile([H, 2 * G3], F32, tag="whhcn")
            nc.sync.dma_start(out=whhcn_sb[:], in_=whhcn_h[:])
            tok_sb = const.tile([128, 4], I32, tag="tok")
            nc.sync.dma_start(out=tok_sb[:], in_=tok_h[:].rearrange("(g p) -> p g", p=128))

            woutc = hall.tile([128, V], BF16, tag="woutc")

            xt = const.tile([E + 1, T], F32, tag="xt")
            nc.vector.memset(xt[E:E + 1, :], 1.0)
            xtr = const.tile([E + 1, T], F32, tag="xtr")

            # Compact input-gate tensors, both direction blocks side by side.
            # Gate order is [z, r, n] so that tz lands at base partition 0
            # (BIR requires equal base partitions for two-SBUF-input ops).
            # gxc rows 0:64 = z/r gx (pad cols: z=+30, r=0); rows 64:96 = 0.
            # xnc = 2 * n-gate gx (pad cols 0).
            gxc = const.tile([G3, 2 * GXW], F32, tag="gxc")
            xnc = const.tile([H, 2 * GXW], F32, tag="xnc")
            for d in range(2):
                o = d * GXW
                nc.vector.memset(gxc[0:H, o:o + 2 * W], 30.0)
                nc.vector.memset(gxc[H:2 * H, o:o + 2 * W], 0.0)
                nc.vector.memset(xnc[:, o:o + 2 * W], 0.0)
            nc.vector.memset(gxc[2 * H:G3, :], 0.0)

            # h state by position, 32 partitions: cols [0, HBW) = L block,
            # [HBW, 2*HBW) = R block; real cols [2W, 2W+2S) within each block.
            hbufp = const.tile([H, 2 * HBW], F32, tag="hbufp")
            nc.vector.memset(hbufp[:], 0.0)

            # rings for zh' ([h;ones]), cn, cz, rz, n
            zhr, cnr, czr, rzr, nnr = [], [], [], [], []
            for j in range(3):
                zt = const.tile([H + 1, LAN], F32, tag=f"zh{j}", name=f"zh{j}")
                nc.vector.memset(zt[0:H, :], 0.0)
                nc.vector.memset(zt[H:H + 1, :], 1.0)
                zhr.append(zt)
                ct = const.tile([H, LAN], F32, tag=f"cn{j}", name=f"cn{j}")
                nc.vector.memset(ct[:], 0.0)
                cnr.append(ct)
                czt = const.tile([H, LAN], F32, tag=f"cz{j}", name=f"cz{j}")
                czr.append(czt)
                rzt = const.tile([2 * H, LAN], F32, tag=f"rz{j}", name=f"rz{j}")
                rzr.append(rzt)
                nnt = const.tile([H, LAN], F32, tag=f"nn{j}", name=f"nn{j}")
                nnr.append(nnt)

            with (
                tc.tile_pool(name="gath", bufs=2) as gpool,
                tc.tile_pool(name="pps", bufs=2, space="PSUM") as ppre,
            ):
                if "pre" in phases:
                    # embedding gather + transpose to [E, tokens]
                    for g in range(4):
                        xg = gpool.tile([128, E], F32, tag="xg")
                        nc.gpsimd.indirect_dma_start(
                            out=xg[:], out_offset=None, in_=emb_h[:],
                            in_offset=bass.IndirectOffsetOnAxis(ap=tok_sb[:, g:g + 1], axis=0),
                        )
                        xps = ppre.tile([E, 128], F32, tag="ps")
                        nc.tensor.transpose(xps[:], xg[:], ident[:])
                        nc.scalar.copy(out=xt[0:E, g * 128:(g + 1) * 128], in_=xps[:])

                    # time-reversed copy (pairwise: s reversed, b kept)
                    src = xt[:]
                    nc.vector.tensor_copy(
                        out=xtr[:],
                        in_=ap3(src, 2 * (S - 1), [[-2, S], [1, 2]]),
                    )

                    # input-gate matmuls -> compact tiles
                    for d, rhs in ((0, xt), (1, xtr)):
                        o = d * GXW
                        prz = ppre.tile([2 * H, T], F32, tag="prz")
                        nc.tensor.matmul(prz[:], wih_sb[:, d * G3:d * G3 + 2 * H], rhs[:],
                                         start=True, stop=True)
                        nc.vector.tensor_copy(out=gxc[0:2 * H, o + 2 * W:o + 2 * W + T], in_=prz[:])
                        pn = ppre.tile([H, T], F32, tag="pn")
                        nc.tensor.matmul(pn[:], wih_sb[:, d * G3 + 2 * H:(d + 1) * G3], rhs[:],
                                         start=True, stop=True)
                        nc.vector.tensor_copy(out=xnc[:, o + 2 * W:o + 2 * W + T], in_=pn[:])

                # wout cache DMA after the gather/weight DMAs (the DMA engine
                # pool is serial; this big load overlaps the scan).
                for c0 in range(0, V, 8192):
                    cw = min(8192, V - c0)
                    nc.sync.dma_start(out=woutc[:, c0:c0 + cw], in_=wout_h[:][:, c0:c0 + cw])

            # ---- chunked fused scan ----
            with tc.tile_pool(name="scp", bufs=3, space="PSUM") as scp:
                for t in range(STEPS if "scan" in phases else 0):
                    pt = scp.tile([128, 2 * LAN], F32, tag="sp")
                    gh = pt[0:G3, 0:LAN]
                    uu = pt[0:H, LAN:2 * LAN]
                    zhp = zhr[(t - 1) % 3]
                    cnp = cnr[(t - 1) % 3]
                    zhc = zhr[t % 3]
                    cnc = cnr[t % 3]
                    czc = czr[t % 3]
                    rzv = rzr[t % 3][:]
                    nnv = nnr[t % 3][:]

                    # gates(t) = gx(t) + 0.5*Whh @ zh'(t-1) + Whh @ cn(t-1)
                    gsrc = ap3(gxc[0:G3, :], 2 * t, [[GXW, 2], [2 * P, C], [1, 2]])
                    nc.tensor.matmul(gh, ident[0:G3, 0:G3], gsrc,
                                     start=True, stop=False, skip_group_check=True)
                    for d in range(2):
                        nc.tensor.matmul(
                            gh[:, d * LPD:(d + 1) * LPD],
                            whhdd_sb[:, d * G3:(d + 1) * G3], zhp[:, d * LPD:(d + 1) * LPD],
                            start=False, stop=False, skip_group_check=True)
                    for d in range(2):
                        nc.tensor.matmul(
                            gh[:, d * LPD:(d + 1) * LPD],
                            whhcn_sb[:, d * G3:(d + 1) * G3], cnp[:, d * LPD:(d + 1) * LPD],
                            start=False, stop=(d == 1), skip_group_check=True)

                    # z,r = sigmoid = .5 + .5*tanh(x/2)  (gate order [z, r, n])
                    nc.scalar.activation(out=rzv, in_=pt[0:2 * H, 0:LAN], func=AF.Tanh, scale=0.5)

                    # zh'(t) = (tz+1)*h(t-1)   (skip at t=0: ring holds zeros)
                    if t > 0:
                        hl = ap3(hbufp[0:H, :], 2 * (t - 1), [[2 * P, C], [1, 2]])
                        nc.vector.scalar_tensor_tensor(
                            out=zhc[0:H, 0:LPD], in0=rzv[0:H, 0:LPD],
                            scalar=1.0, in1=hl, op0=ALU.add, op1=ALU.mult)
                        hr = ap3(hbufp[0:H, :], HBW + (4 * W + 2 * S - 2) - 2 * (t - 1),
                                 [[-2 * P, C], [1, 2]])
                        nc.vector.scalar_tensor_tensor(
                            out=zhc[0:H, LPD:LAN], in0=rzv[0:H, LPD:LAN],
                            scalar=1.0, in1=hr, op0=ALU.add, op1=ALU.mult)
                    # cz = (1-z) = .5 - .5*tz
                    nc.gpsimd.tensor_scalar(czc[:], rzv[0:H, :], -0.5, 0.5,
                                            ALU.mult, ALU.add)

                    # n path: u = (tr+1)*hn + 2*xn ; n = tanh(u/2)
                    nc.vector.scalar_tensor_tensor(
                        out=uu, in0=rzv[H:2 * H, :], scalar=1.0,
                        in1=pt[2 * H:G3, 0:LAN], op0=ALU.add, op1=ALU.mult)
                    usrc = ap3(xnc[0:H, :], 2 * t, [[GXW, 2], [2 * P, C], [1, 2]])
                    nc.vector.tensor_tensor(uu, uu, usrc, ALU.add)
                    nc.scalar.activation(out=nnv, in_=uu, func=AF.Tanh, scale=0.5)

                    # cn = (1-z)*n ; h(t) = cn + 0.5*zh'
                    nc.gpsimd.tensor_tensor(cnc[:], nnv, czc[:], ALU.mult)
                    hl = ap3(hbufp[0:H, :], 2 * t, [[2 * P, C], [1, 2]])
                    nc.vector.scalar_tensor_tensor(
                        out=hl, in0=zhc[0:H, 0:LPD], scalar=0.5, in1=cnc[:, 0:LPD],
                        op0=ALU.mult, op1=ALU.add)
                    hr = ap3(hbufp[0:H, :], HBW + (4 * W + 2 * S - 2) - 2 * t,
                             [[-2 * P, C], [1, 2]])
                    nc.vector.scalar_tensor_tensor(
                        out=hr, in0=zhc[0:H, LPD:LAN], scalar=0.5, in1=cnc[:, LPD:LAN],
                        op0=ALU.mult, op1=ALU.add)

            # ---- pack shells (bf16, 128 tokens):
            # rows [L-hi 0:32 | R-hi 32:64 | L-lo 64:96 | R-lo 96:127 | ones 127]
            hsh = []
            for k in range(4):
                hs = hall.tile([128, 128], BF16, tag=f"hs{k}", name=f"hs{k}")
                cl = 2 * W + 128 * k
                cr = HBW + 2 * W + 128 * k
                nc.vector.tensor_copy(out=hs[0:H, :], in_=hbufp[:, cl:cl + 128])
                nc.vector.tensor_copy(out=hs[H:2 * H, :], in_=hbufp[:, cr:cr + 128])
                nc.vector.tensor_tensor(hs[2 * H:G3, :], hbufp[:, cl:cl + 128],
                                        hs[0:H, :], ALU.subtract)
                hr16 = hall.tile([H, 128], BF16, tag=f"hr{k}", name=f"hr16_{k}")
                nc.vector.tensor_copy(out=hr16[:], in_=hbufp[:, cr:cr + 128])
                nc.vector.tensor_tensor(hr16[0:H - 1, :], hbufp[0:H - 1, cr:cr + 128],
                                        hr16[0:H - 1, :], ALU.subtract)
                nc.vector.tensor_copy(out=hs[G3:127, :], in_=hr16[0:H - 1, :])
                nc.sync.dma_start(out=hs[127:128, :], in_=ones_h[:])
                hsh.append(hs)

            do_proj = "proj" in phases
            if not do_proj and "scan" not in phases:
                for k in range(4):
                    nc.vector.memset(hsh[k][:], 0.0)

            # ---- projection + log-softmax ----
            with (
                tc.tile_pool(name="pp", bufs=NPSB, space="PSUM") as ppool,
                tc.tile_pool(name="esc", bufs=2) as epool,
                tc.tile_pool(name="outp", bufs=4) as opool,
            ):
                stats = [const.tile([128, NGRP], F32, tag=f"st{k}", name=f"stats{k}")
                         for k in range(4)]
                negc = [const.tile([128, 1], F32, tag=f"ng{k}", name=f"negc{k}")
                        for k in range(4)]

                def groups():
                    for g in range(NGRP):
                        c0 = g * VGRP
                        yield g, c0, min(VGRP, V - c0)

                def mms(k, g, c0, gw, tag):
                    ps = ppool.tile([128, VGRP], F32, tag="pp", name=f"pp{tag}{k}_{g}")
                    for q0 in range(0, gw, 512):
                        qw = min(512, gw - q0)
                        nc.tensor.matmul(ps[:, q0:q0 + qw], hsh[k][:],
                                         woutc[:, c0 + q0:c0 + q0 + qw],
                                         start=True, stop=True)
                    return ps

                def p1(k, g, c0, gw):
                    ps = mms(k, g, c0, gw, "a")
                    esc = epool.tile([128, VGRP], BF16, tag="esc", name=f"esc{k}_{g}")
                    nc.scalar.activation(out=esc[:, 0:gw], in_=ps[:, 0:gw], func=AF.Exp,
                                         accum_out=stats[k][:, g:g + 1])

                def lse(k):
                    ssum = const.tile([128, 1], F32, tag=f"ss{k}", name=f"ssum{k}")
                    nc.vector.tensor_reduce(out=ssum[:], in_=stats[k][:],
                                            axis=mybir.AxisListType.X, op=ALU.add)
                    nc.scalar.activation(out=negc[k][:], in_=ssum[:], func=AF.Ln)
                    nc.vector.tensor_scalar_mul(negc[k][:], negc[k][:], -1.0)

                def p2(k, g, c0, gw, split=None):
                    ps = mms(k, g, c0, gw, "b")
                    ob = opool.tile([128, VGRP], BF16, tag="ob", name=f"ob{k}_{g}")
                    split = fin_split if split is None else split
                    eng = split[g % len(split)]
                    if eng == "v":
                        nc.vector.tensor_scalar_add(ob[:, 0:gw], ps[:, 0:gw], negc[k][:, 0:1])
                    else:
                        nc.scalar.activation(out=ob[:, 0:gw], in_=ps[:, 0:gw],
                                             func=AF.Identity, bias=negc[k][:, 0:1])
                    dst = bass.AP(tensor=out_h[:].tensor, offset=(128 * k) * V + c0,
                                  ap=[[V, 128], [1, gw]])
                    nc.sync.dma_start(out=dst, in_=ob[:, 0:gw])

                if do_proj:
                    for g, c0, gw in groups():
                        p1(0, g, c0, gw)
                    for k in range(4):
                        lse(k)
                        if k < 3:
                            for g, c0, gw in groups():
                                p1(k + 1, g, c0, gw)
                                p2(k, g, c0, gw)
                        else:
                            # last shell: no pass-1 to overlap; split the
                            # finalize adds across ACT and DVE evenly.
                            for g, c0, gw in groups():
                                p2(k, g, c0, gw, split=("a", "v"))
    nc.compile()
    return nc


_CACHE = {}


def _get_module():
    if "nc" not in _CACHE:
        _CACHE["nc"] = build_module()
    return _CACHE["nc"]


def prep_inputs(inputs):
    """Host-side prep: build per-core input maps from the full input dict."""
    import ml_dtypes

    ib = np.asarray(inputs["input_batch"])
    embed = np.ascontiguousarray(np.asarray(inputs["embed"], dtype=np.float32))

    def f32(x):
        return np.asarray(x, dtype=np.float32)

    # Gate order [z, r, n] (see kernel); n-gate input weights pre-doubled.
    perm = np.concatenate([np.arange(H, 2 * H), np.arange(0, H),
                           np.arange(2 * H, G3)])
    nsc = np.concatenate([np.ones(2 * H, np.float32), 2.0 * np.ones(H, np.float32)])

    wih = np.zeros((E + 1, 2 * G3), np.float32)
    for d, (Wd, bd) in enumerate(((inputs["Wl_ih"], inputs["bl_ih"]),
                                  (inputs["Wr_ih"], inputs["br_ih"]))):
        Wd, bd = f32(Wd), f32(bd)
        o = d * G3
        wih[:E, o:o + G3] = Wd[:, perm] * nsc
        wih[E, o:o + G3] = bd[perm] * nsc

    whhdd = np.zeros((H + 1, 2 * G3), np.float32)
    whhcn = np.zeros((H, 2 * G3), np.float32)
    for d, (Wd, bd) in enumerate(((inputs["Wl_hh"], inputs["bl_hh"]),
                                  (inputs["Wr_hh"], inputs["br_hh"]))):
        Wd, bd = f32(Wd), f32(bd)
        o = d * G3
        whhdd[:H, o:o + G3] = 0.5 * Wd[:, perm]
        whhdd[H, o:o + G3] = bd[perm]
        whhcn[:, o:o + G3] = Wd[:, perm]

    rnn_out = f32(inputs["rnn_out"])
    rnn_out_bias = f32(inputs["rnn_out_bias"])
    wout = np.zeros((128, V), np.float32)
    wout[0:2 * H] = rnn_out                  # hi rows (L then R)
    wout[2 * H:127] = rnn_out[0:2 * H - 1]   # lo rows (L 32, R 31)
    wout[127] = rnn_out_bias[0]              # ones row
    woutp = wout.astype(ml_dtypes.bfloat16)

    in_maps = []
    for c in range(NCORES):
        tok = np.ascontiguousarray(
            ib[:, BC * c:BC * (c + 1)].astype(np.int32).reshape(T)
        )
        in_maps.append(
            {"tok": tok, "embed": embed, "wih": wih, "whhdd": whhdd,
             "whhcn": whhcn, "wout": woutp,
             "onesrow": np.ones((1, 128), ml_dtypes.bfloat16)}
        )
    return in_maps


def assemble_output(results):
    out = np.empty((S, B, V), np.float32)
    for c in range(NCORES):
        out[:, BC * c:BC * (c + 1), :] = (
            results[c]["out"].astype(np.float32).reshape(S, BC, V)
        )
    return out


def kernel(**inputs):
    from concourse.bass_utils import run_bass_kernel_spmd

    nc = _get_module()
    in_maps = prep_inputs(inputs)
    res = run_bass_kernel_spmd(nc, in_maps, core_ids=list(range(NCORES)))
    return assemble_output(res.results)


# revision 44
# speedup vs baseline: 2.5717x; 1.0621x over previous
"""BiRNN (bidirectional GRU) language model kernel for Trainium2, 8 NeuronCores.

Sharding: data-parallel over batch (2 of 16 batch columns per core), no
collectives.  Each core computes the embedding gather, both GRU scans, the
vocab projection and the log-softmax for its 512 tokens.

Key structure:
  - Chunked scan: each direction's 256-step recurrence is split into C chunks
    run in parallel as lanes of the same instructions, each with a W-step
    warmup ramp.  Positions before the sequence start are padded with a
    frozen-gate column (z-gate pre-activation +30 => z=1 => h stays exactly 0),
    so chunks whose warmup crosses position 0 are exact and later chunks
    converge through the GRU's state contraction (validated: W=32 gives
    ~5e-7 max h error).  Serial steps: 256 -> 256/C + W.
  - Per step, the gate matmul Whh @ h(t-1) is split into an accumulation of
    three PSUM matmuls: identity @ gx(t) (the precomputed input gates),
    0.5*Whh @ zh'(t-1) and Whh @ cn(t-1), where h = cn + 0.5*zh'
    (cn = (1-z)*n, zh' = 2z*h).  This keeps the recurrence's serial path at
    mm -> tanh(rz) -> (r+1)*hn -> +xn -> tanh -> *(1-z) -> mm.
  - h states land in a position-indexed buffer (later chunks' warmup writes
    are overwritten by the owning chunk's real writes, which come later in
    step order).
  - Vocab projection: h split bf16 hi/lo packed into one K=128 stationary
    tile per 128 tokens ([h-hi 64 | ones | h-lo 63]); wout packed to match
    ([w-hi 64 | bias | w-hi 63]) so one bf16 matmul per 512 columns gives
    logits to ~2e-3 abs.  wout ([128, V] bf16, 12.9MB) is fully cached in
    SBUF, loaded during the scan.
  - log-softmax without a max pass (|logits| <= 65 so exp can't overflow):
    pass 1 Exp+accum_out per 2048-col group; pass 2 recomputes logits and
    writes logits - log(sum) as bf16 (host converts to f32).  Pass 2 of
    shell k is interleaved with pass 1 of shell k+1; the finalize add is
    split across DVE and Pool so ACT only does the Exps.
"""

import os
import sys
from contextlib import ExitStack

import numpy as np

for _p in (
    "/opt/trn_rl_repo",
    "/root/.axon_site",
    "/root/.axon_site/_ro/trn_rl_repo",
    "/root/.axon_site/_ro/pypackages",
):
    if os.path.isdir(_p) and _p not in sys.path:
        sys.path.append(_p)

import concourse.bass as bass
import concourse.bacc as bacc
import concourse.tile as tile
from concourse import mybir
from concourse.masks import make_identity

F32 = mybir.dt.float32
BF16 = mybir.dt.bfloat16
I32 = mybir.dt.int32
AF = mybir.ActivationFunctionType
ALU = mybir.AluOpType

V = 50257
E = 64
H = 32
S = 256
B = 16
NCORES = 8
BC = B // NCORES          # batch columns per core
T = S * BC                # tokens per core
G3 = 3 * H                # 96 gate rows (r, z, n)

C = int(os.environ.get("KC", "32"))    # chunks per direction
W = int(os.environ.get("KW", "24"))    # warmup steps
P = S // C                              # positions per chunk
STEPS = P + W
LPD = 2 * C                             # lanes per direction (chunk x batchcol)
LAN = 2 * LPD                           # total lanes (L block, R block)
GXW = 2 * W + 2 * S                     # compact gx width per direction block
HBW = 2 * S + 4 * W                     # h buffer width (L pad left, R pad right)

VGRP = int(os.environ.get("KVGRP", "1024"))  # vocab columns per projection group
NPSB = int(os.environ.get("KNPSB", "4"))     # projection PSUM buffers
NGRP = (V + VGRP - 1) // VGRP


def ap3(base, offset, dims):
    """AP with the partition entry of ``base`` plus custom free dims."""
    return bass.AP(tensor=base.tensor, offset=base.offset + offset,
                   ap=[list(base.ap[0])] + [list(d) for d in dims])


def build_module(phases=("pre", "scan", "proj"),
                 fin_split=tuple(os.environ.get("KFIN", "v"))):
    nc = bacc.Bacc("TRN2", target_bir_lowering=False)
    tok_h = nc.dram_tensor("tok", (T,), I32, kind="ExternalInput")
    emb_h = nc.dram_tensor("embed", (V, E), F32, kind="ExternalInput")
    wih_h = nc.dram_tensor("wih", (E + 1, 2 * G3), F32, kind="ExternalInput")
    whhdd_h = nc.dram_tensor("whhdd", (H + 1, 2 * G3), F32, kind="ExternalInput")
    whhcn_h = nc.dram_tensor("whhcn", (H, 2 * G3), F32, kind="ExternalInput")
    wout_h = nc.dram_tensor("wout", (128, V), BF16, kind="ExternalInput")
    ones_h = nc.dram_tensor("onesrow", (1, 128), BF16, kind="ExternalInput")
    out_h = nc.dram_tensor("out", (T, V), BF16, kind="ExternalOutput")

    with tile.TileContext(nc) as tc:
        with ExitStack() as ctx:
            const = ctx.enter_context(tc.tile_pool(name="const", bufs=1))
            hall = ctx.enter_context(tc.tile_pool(name="hall", bufs=1))

            ident = const.tile([128, 128], F32, tag="ident")
            make_identity(nc, ident[:])
            wih_sb = const.tile([E + 1, 2 * G3], F32, tag="wih")
            nc.sync.dma_start(out=wih_sb[:], in_=wih_h[:])
            whhdd_sb = const.tile([H + 1, 2 * G3], F32, tag="whhdd")
            nc.sync.dma_start(out=whhdd_sb[:], in_=whhdd_h[:])
            whhcn_sb = const.tile([H, 2 * G3], F32, tag="whhcn")
            nc.sync.dma_start(out=whhcn_sb[:], in_=whhcn_h[:])
            tok_sb = const.tile([128, 4], I32, tag="tok")
            nc.sync.dma_start(out=tok_sb[:], in_=tok_h[:].rearrange("(g p) -> p g", p=128))

            woutc = hall.tile([128, V], BF16, tag="woutc")

            xt = const.tile([E + 1, T], F32, tag="xt")
            nc.vector.memset(xt[E:E + 1, :], 1.0)
            xtr = const.tile([E + 1, T], F32, tag="xtr")

            # Compact input-gate tensors, both direction blocks side by side.
            # Gate order is [z, r, n] so that tz lands at base partition 0
            # (BIR requires equal base partitions for two-SBUF-input ops).
            # gxc rows 0:64 = z/r gx (pad cols: z=+30, r=0); rows 64:96 = 0.
            # xnc = 2 * n-gate gx (pad cols 0).
            gxc = const.tile([G3, 2 * GXW], F32, tag="gxc")
            xnc = const.tile([H, 2 * GXW], F32, tag="xnc")
            for d in range(2):
                o = d * GXW
                nc.vector.memset(gxc[0:H, o:o + 2 * W], 30.0)
                nc.vector.memset(gxc[H:2 * H, o:o + 2 * W], 0.0)
                nc.vector.memset(xnc[:, o:o + 2 * W], 0.0)
            nc.vector.memset(gxc[2 * H:G3, :], 0.0)

            # h state by position, 32 partitions: cols [0, HBW) = L block,
            # [HBW, 2*HBW) = R block; real cols [2W, 2W+2S) within each block.
            hbufp = const.tile([H, 2 * HBW], F32, tag="hbufp")
            nc.vector.memset(hbufp[:], 0.0)

            # rings for zh' ([h;ones]), cn, cz, rz, n
            zhr, cnr, czr, rzr, nnr = [], [], [], [], []
            for j in range(3):
                zt = const.tile([H + 1, LAN], F32, tag=f"zh{j}", name=f"zh{j}")
                nc.vector.memset(zt[0:H, :], 0.0)
                nc.vector.memset(zt[H:H + 1, :], 1.0)
                zhr.append(zt)
                ct = const.tile([H, LAN], F32, tag=f"cn{j}", name=f"cn{j}")
                nc.vector.memset(ct[:], 0.0)
                cnr.append(ct)
                czt = const.tile([H, LAN], F32, tag=f"cz{j}", name=f"cz{j}")
                czr.append(czt)
                rzt = const.tile([2 * H, LAN], F32, tag=f"rz{j}", name=f"rz{j}")
                rzr.append(rzt)
                nnt = const.tile([H, LAN], F32, tag=f"nn{j}", name=f"nn{j}")
                nnr.append(nnt)
            uur = []
            for j in range(3):
                uut = const.tile([H, LAN], F32, tag=f"uu{j}", name=f"uu{j}")
                uur.append(uut)

            with (
                tc.tile_pool(name="gath", bufs=2) as gpool,
                tc.tile_pool(name="pps", bufs=2, space="PSUM") as ppre,
            ):
                if "pre" in phases:
                    # embedding gather + transpose to [E, tokens]
                    for g in range(4):
                        xg = gpool.tile([128, E], F32, tag="xg")
                        nc.gpsimd.indirect_dma_start(
                            out=xg[:], out_offset=None, in_=emb_h[:],
                            in_offset=bass.IndirectOffsetOnAxis(ap=tok_sb[:, g:g + 1], axis=0),
                        )
                        xps = ppre.tile([E, 128], F32, tag="ps")
                        nc.tensor.transpose(xps[:], xg[:], ident[:])
                        nc.scalar.copy(out=xt[0:E, g * 128:(g + 1) * 128], in_=xps[:])

                    # time-reversed copy (pairwise: s reversed, b kept)
                    src = xt[:]
                    nc.vector.tensor_copy(
                        out=xtr[:],
                        in_=ap3(src, 2 * (S - 1), [[-2, S], [1, 2]]),
                    )

                    # input-gate matmuls -> compact tiles
                    for d, rhs in ((0, xt), (1, xtr)):
                        o = d * GXW
                        prz = ppre.tile([2 * H, T], F32, tag="prz")
                        nc.tensor.matmul(prz[:], wih_sb[:, d * G3:d * G3 + 2 * H], rhs[:],
                                         start=True, stop=True)
                        nc.vector.tensor_copy(out=gxc[0:2 * H, o + 2 * W:o + 2 * W + T], in_=prz[:])
                        pn = ppre.tile([H, T], F32, tag="pn")
                        nc.tensor.matmul(pn[:], wih_sb[:, d * G3 + 2 * H:(d + 1) * G3], rhs[:],
                                         start=True, stop=True)
                        nc.vector.tensor_copy(out=xnc[:, o + 2 * W:o + 2 * W + T], in_=pn[:])

                # wout cache DMA after the gather/weight DMAs; small chunks so
                # the embed gathers interleave on the serial DMA-engine pool.
                for c0 in range(0, V, 2048):
                    cw = min(2048, V - c0)
                    nc.sync.dma_start(out=woutc[:, c0:c0 + cw], in_=wout_h[:][:, c0:c0 + cw])

            # ---- chunked fused scan ----
            with tc.tile_pool(name="scp", bufs=3, space="PSUM") as scp:
                for t in range(STEPS if "scan" in phases else 0):
                    pt = scp.tile([128, LAN], F32, tag="sp")
                    gh = pt[0:G3, 0:LAN]
                    uu = uur[t % 3][:]
                    zhp = zhr[(t - 1) % 3]
                    cnp = cnr[(t - 1) % 3]
                    zhc = zhr[t % 3]
                    cnc = cnr[t % 3]
                    czc = czr[t % 3]
                    rzv = rzr[t % 3][:]
                    nnv = nnr[t % 3][:]

                    # gates(t) = gx(t) + 0.5*Whh @ zh'(t-1) + Whh @ cn(t-1)
                    gsrc = ap3(gxc[0:G3, :], 2 * t, [[GXW, 2], [2 * P, C], [1, 2]])
                    nc.tensor.matmul(gh, ident[0:G3, 0:G3], gsrc,
                                     start=True, stop=False, skip_group_check=True)
                    for d in range(2):
                        nc.tensor.matmul(
                            gh[:, d * LPD:(d + 1) * LPD],
                            whhdd_sb[:, d * G3:(d + 1) * G3], zhp[:, d * LPD:(d + 1) * LPD],
                            start=False, stop=False, skip_group_check=True)
                    for d in range(2):
                        nc.tensor.matmul(
                            gh[:, d * LPD:(d + 1) * LPD],
                            whhcn_sb[:, d * G3:(d + 1) * G3], cnp[:, d * LPD:(d + 1) * LPD],
                            start=False, stop=(d == 1), skip_group_check=True)

                    # z,r = sigmoid = .5 + .5*tanh(x/2)  (gate order [z, r, n])
                    nc.scalar.activation(out=rzv, in_=pt[0:2 * H, 0:LAN], func=AF.Tanh, scale=0.5)

                    # n path first in the DVE stream (critical):
                    # u = (tr+1)*hn + 2*xn ; n = tanh(u/2)
                    nc.vector.scalar_tensor_tensor(
                        out=uu, in0=rzv[H:2 * H, :], scalar=1.0,
                        in1=pt[2 * H:G3, 0:LAN], op0=ALU.add, op1=ALU.mult)
                    usrc = ap3(xnc[0:H, :], 2 * t, [[GXW, 2], [2 * P, C], [1, 2]])
                    nc.vector.tensor_tensor(uu, uu, usrc, ALU.add)
                    nc.scalar.activation(out=nnv, in_=uu, func=AF.Tanh, scale=0.5)

                    # off-path: zh'(t) = (tz+1)*h(t-1) (skip at t=0: ring
                    # holds zeros), cz = (1-z) = .5 - .5*tz (Pool)
                    if t > 0:
                        hl = ap3(hbufp[0:H, :], 2 * (t - 1), [[2 * P, C], [1, 2]])
                        nc.vector.scalar_tensor_tensor(
                            out=zhc[0:H, 0:LPD], in0=rzv[0:H, 0:LPD],
                            scalar=1.0, in1=hl, op0=ALU.add, op1=ALU.mult)
                        hr = ap3(hbufp[0:H, :], HBW + (4 * W + 2 * S - 2) - 2 * (t - 1),
                                 [[-2 * P, C], [1, 2]])
                        nc.vector.scalar_tensor_tensor(
                            out=zhc[0:H, LPD:LAN], in0=rzv[0:H, LPD:LAN],
                            scalar=1.0, in1=hr, op0=ALU.add, op1=ALU.mult)
                    nc.gpsimd.tensor_scalar(czc[:], rzv[0:H, :], -0.5, 0.5,
                                            ALU.mult, ALU.add)

                    # cn = (1-z)*n ; h(t) = cn + 0.5*zh'
                    nc.vector.tensor_tensor(cnc[:], nnv, czc[:], ALU.mult)
                    hl = ap3(hbufp[0:H, :], 2 * t, [[2 * P, C], [1, 2]])
                    nc.vector.scalar_tensor_tensor(
                        out=hl, in0=zhc[0:H, 0:LPD], scalar=0.5, in1=cnc[:, 0:LPD],
                        op0=ALU.mult, op1=ALU.add)
                    hr = ap3(hbufp[0:H, :], HBW + (4 * W + 2 * S - 2) - 2 * t,
                             [[-2 * P, C], [1, 2]])
                    nc.vector.scalar_tensor_tensor(
                        out=hr, in0=zhc[0:H, LPD:LAN], scalar=0.5, in1=cnc[:, LPD:LAN],
                        op0=ALU.mult, op1=ALU.add)

            # ---- pack shells (bf16, 128 tokens):
            # rows [L-hi 0:32 | R-hi 32:64 | L-lo 64:96 | R-lo 96:127 | ones 127]
            hsh = []
            for k in range(4):
                hs = hall.tile([128, 128], BF16, tag=f"hs{k}", name=f"hs{k}")
                cl = 2 * W + 128 * k
                cr = HBW + 2 * W + 128 * k
                nc.vector.tensor_copy(out=hs[0:H, :], in_=hbufp[:, cl:cl + 128])
                nc.vector.tensor_copy(out=hs[H:2 * H, :], in_=hbufp[:, cr:cr + 128])
                nc.vector.tensor_tensor(hs[2 * H:G3, :], hbufp[:, cl:cl + 128],
                                        hs[0:H, :], ALU.subtract)
                hr16 = hall.tile([H, 128], BF16, tag=f"hr{k}", name=f"hr16_{k}")
                nc.vector.tensor_copy(out=hr16[:], in_=hbufp[:, cr:cr + 128])
                nc.vector.tensor_tensor(hr16[0:H - 1, :], hbufp[0:H - 1, cr:cr + 128],
                                        hr16[0:H - 1, :], ALU.subtract)
                nc.vector.tensor_copy(out=hs[G3:127, :], in_=hr16[0:H - 1, :])
                nc.sync.dma_start(out=hs[127:128, :], in_=ones_h[:])
                hsh.append(hs)

            do_proj = "proj" in phases
            if not do_proj and "scan" not in phases:
                for k in range(4):
                    nc.vector.memset(hsh[k][:], 0.0)

            # ---- projection + log-softmax ----
            opool = ctx.enter_context(tc.tile_pool(name="outp", bufs=4))
            with (
                tc.tile_pool(name="pp", bufs=NPSB, space="PSUM") as ppool,
                tc.tile_pool(name="esc", bufs=6) as epool,
            ):
                stats = [const.tile([128, NGRP], F32, tag=f"st{k}", name=f"stats{k}")
                         for k in range(4)]
                negc = [const.tile([128, 1], F32, tag=f"ng{k}", name=f"negc{k}")
                        for k in range(4)]

                def groups():
                    for g in range(NGRP):
                        c0 = g * VGRP
                        yield g, c0, min(VGRP, V - c0)

                def mms(k, g, c0, gw, tag):
                    ps = ppool.tile([128, VGRP], F32, tag="pp", name=f"pp{tag}{k}_{g}")
                    for q0 in range(0, gw, 512):
                        qw = min(512, gw - q0)
                        nc.tensor.matmul(ps[:, q0:q0 + qw], hsh[k][:],
                                         woutc[:, c0 + q0:c0 + q0 + qw],
                                         start=True, stop=True)
                    return ps

                def p1(k, g, c0, gw, pool_red=False):
                    ps = mms(k, g, c0, gw, "a")
                    esc = epool.tile([128, VGRP], BF16, tag="esc", name=f"esc{k}_{g}")
                    if pool_red:
                        # sum on Pool from the bf16 exp scratch; saves the
                        # ACT accumulator-read overhead on the critical engine
                        nc.scalar.activation(out=esc[:, 0:gw], in_=ps[:, 0:gw], func=AF.Exp)
                        nc.gpsimd.tensor_scalar(esc[:, 0:gw], esc[:, 0:gw], 1.0, 0.0,
                                                ALU.mult, ALU.add,
                                                accum_out=stats[k][:, g:g + 1])
                    else:
                        nc.scalar.activation(out=esc[:, 0:gw], in_=ps[:, 0:gw], func=AF.Exp,
                                             accum_out=stats[k][:, g:g + 1])

                def lse(k):
                    ssum = const.tile([128, 1], F32, tag=f"ss{k}", name=f"ssum{k}")
                    nc.vector.tensor_reduce(out=ssum[:], in_=stats[k][:],
                                            axis=mybir.AxisListType.X, op=ALU.add)
                    nc.scalar.activation(out=negc[k][:], in_=ssum[:], func=AF.Ln)
                    nc.vector.tensor_scalar_mul(negc[k][:], negc[k][:], -1.0)

                def p2(k, g, c0, gw, split=None):
                    ps = mms(k, g, c0, gw, "b")
                    ob = opool.tile([128, VGRP], BF16, tag="ob", name=f"ob{k}_{g}")
                    if split == "half":
                        # drain mode: ACT and DVE each finalize half the tile
                        h0 = (gw + 1) // 2
                        nc.scalar.activation(out=ob[:, 0:h0], in_=ps[:, 0:h0],
                                             func=AF.Identity, bias=negc[k][:, 0:1])
                        nc.vector.tensor_scalar_add(ob[:, h0:gw], ps[:, h0:gw],
                                                    negc[k][:, 0:1])
                    else:
                        split = fin_split if split is None else split
                        eng = split[g % len(split)]
                        if eng == "v":
                            nc.vector.tensor_scalar_add(ob[:, 0:gw], ps[:, 0:gw],
                                                        negc[k][:, 0:1])
                        else:
                            nc.scalar.activation(out=ob[:, 0:gw], in_=ps[:, 0:gw],
                                                 func=AF.Identity, bias=negc[k][:, 0:1])
                    dst = bass.AP(tensor=out_h[:].tensor, offset=(128 * k) * V + c0,
                                  ap=[[V, 128], [1, gw]])
                    nc.sync.dma_start(out=dst, in_=ob[:, 0:gw])

                if do_proj:
                    for g, c0, gw in groups():
                        p1(0, g, c0, gw)
                    for k in range(4):
                        lse(k)
                        if k < 3:
                            for g, c0, gw in groups():
                                p1(k + 1, g, c0, gw)
                                p2(k, g, c0, gw)
                        else:
                            # drain: no pass-1 left; ACT and DVE each take
                            # half of every finalize tile
                            for g, c0, gw in groups():
                                p2(k, g, c0, gw, split="half")
    nc.compile()
    return nc


_CACHE = {}


def _get_module():
    if "nc" not in _CACHE:
        _CACHE["nc"] = build_module()
    return _CACHE["nc"]


def prep_inputs(inputs):
    """Host-side prep: build per-core input maps from the full input dict."""
    import ml_dtypes

    ib = np.asarray(inputs["input_batch"])
    embed = np.ascontiguousarray(np.asarray(inputs["embed"], dtype=np.float32))

    def f32(x):
        return np.asarray(x, dtype=np.float32)

    # Gate order [z, r, n] (see kernel); n-gate input weights pre-doubled.
    perm = np.concatenate([np.arange(H, 2 * H), np.arange(0, H),
                           np.arange(2 * H, G3)])
    nsc = np.concatenate([np.ones(2 * H, np.float32), 2.0 * np.ones(H, np.float32)])

    wih = np.zeros((E + 1, 2 * G3), np.float32)
    for d, (Wd, bd) in enumerate(((inputs["Wl_ih"], inputs["bl_ih"]),
                                  (inputs["Wr_ih"], inputs["br_ih"]))):
        Wd, bd = f32(Wd), f32(bd)
        o = d * G3
        wih[:E, o:o + G3] = Wd[:, perm] * nsc
        wih[E, o:o + G3] = bd[perm] * nsc

    whhdd = np.zeros((H + 1, 2 * G3), np.float32)
    whhcn = np.zeros((H, 2 * G3), np.float32)
    for d, (Wd, bd) in enumerate(((inputs["Wl_hh"], inputs["bl_hh"]),
                                  (inputs["Wr_hh"], inputs["br_hh"]))):
        Wd, bd = f32(Wd), f32(bd)
        o = d * G3
        whhdd[:H, o:o + G3] = 0.5 * Wd[:, perm]
        whhdd[H, o:o + G3] = bd[perm]
        whhcn[:, o:o + G3] = Wd[:, perm]

    rnn_out = f32(inputs["rnn_out"])
    rnn_out_bias = f32(inputs["rnn_out_bias"])
    wout = np.zeros((128, V), np.float32)
    wout[0:2 * H] = rnn_out                  # hi rows (L then R)
    wout[2 * H:127] = rnn_out[0:2 * H - 1]   # lo rows (L 32, R 31)
    wout[127] = rnn_out_bias[0]              # ones row
    woutp = wout.astype(ml_dtypes.bfloat16)

    in_maps = []
    for c in range(NCORES):
        tok = np.ascontiguousarray(
            ib[:, BC * c:BC * (c + 1)].astype(np.int32).reshape(T)
        )
        in_maps.append(
            {"tok": tok, "embed": embed, "wih": wih, "whhdd": whhdd,
             "whhcn": whhcn, "wout": woutp,
             "onesrow": np.ones((1, 128), ml_dtypes.bfloat16)}
        )
    return in_maps


def assemble_output(results):
    out = np.empty((S, B, V), np.float32)
    for c in range(NCORES):
        out[:, BC * c:BC * (c + 1), :] = (
            results[c]["out"].astype(np.float32).reshape(S, BC, V)
        )
    return out


def kernel(**inputs):
    from concourse.bass_utils import run_bass_kernel_spmd

    nc = _get_module()
    in_maps = prep_inputs(inputs)
    res = run_bass_kernel_spmd(nc, in_maps, core_ids=list(range(NCORES)))
    return assemble_output(res.results)


# revision 47
# speedup vs baseline: 2.8861x; 1.1223x over previous
"""BiRNN (bidirectional GRU) language model kernel for Trainium2, 8 NeuronCores.

Sharding: data-parallel over batch (2 of 16 batch columns per core), no
collectives.  Each core computes the embedding gather, both GRU scans, the
vocab projection and the log-softmax for its 512 tokens.

Key structure:
  - Chunked scan: each direction's 256-step recurrence is split into C chunks
    run in parallel as lanes of the same instructions, each with a W-step
    warmup ramp.  Positions before the sequence start are padded with a
    frozen-gate column (z-gate pre-activation +30 => z=1 => h stays exactly 0),
    so chunks whose warmup crosses position 0 are exact and later chunks
    converge through the GRU's state contraction (validated: W=32 gives
    ~5e-7 max h error).  Serial steps: 256 -> 256/C + W.
  - Per step, the gate matmul Whh @ h(t-1) is split into an accumulation of
    three PSUM matmuls: identity @ gx(t) (the precomputed input gates),
    0.5*Whh @ zh'(t-1) and Whh @ cn(t-1), where h = cn + 0.5*zh'
    (cn = (1-z)*n, zh' = 2z*h).  This keeps the recurrence's serial path at
    mm -> tanh(rz) -> (r+1)*hn -> +xn -> tanh -> *(1-z) -> mm.
  - h states land in a position-indexed buffer (later chunks' warmup writes
    are overwritten by the owning chunk's real writes, which come later in
    step order).
  - Vocab projection: h split bf16 hi/lo packed into one K=128 stationary
    tile per 128 tokens ([h-hi 64 | ones | h-lo 63]); wout packed to match
    ([w-hi 64 | bias | w-hi 63]) so one bf16 matmul per 512 columns gives
    logits to ~2e-3 abs.  wout ([128, V] bf16, 12.9MB) is fully cached in
    SBUF, loaded during the scan.
  - log-softmax without a max pass (|logits| <= 65 so exp can't overflow):
    pass 1 Exp+accum_out per 2048-col group; pass 2 recomputes logits and
    writes logits - log(sum) as bf16 (host converts to f32).  Pass 2 of
    shell k is interleaved with pass 1 of shell k+1; the finalize add is
    split across DVE and Pool so ACT only does the Exps.
"""

import os
import sys
from contextlib import ExitStack

import numpy as np

for _p in (
    "/opt/trn_rl_repo",
    "/root/.axon_site",
    "/root/.axon_site/_ro/trn_rl_repo",
    "/root/.axon_site/_ro/pypackages",
):
    if os.path.isdir(_p) and _p not in sys.path:
        sys.path.append(_p)

import concourse.bass as bass
import concourse.bacc as bacc
import concourse.tile as tile
from concourse import mybir
from concourse.masks import make_identity

F32 = mybir.dt.float32
BF16 = mybir.dt.bfloat16
I32 = mybir.dt.int32
AF = mybir.ActivationFunctionType
ALU = mybir.AluOpType

V = 50257
E = 64
H = 32
S = 256
B = 16
NCORES = 8
BC = B // NCORES          # batch columns per core
T = S * BC                # tokens per core
G3 = 3 * H                # 96 gate rows (r, z, n)

C = int(os.environ.get("KC", "32"))    # chunks per direction
W = int(os.environ.get("KW", "24"))    # warmup steps
P = S // C                              # positions per chunk
STEPS = P + W
LPD = 2 * C                             # lanes per direction (chunk x batchcol)
LAN = 2 * LPD                           # total lanes (L block, R block)
GXW = 2 * W + 2 * S                     # compact gx width per direction block
HBW = 2 * S + 4 * W                     # h buffer width (L pad left, R pad right)

VGRP = int(os.environ.get("KVGRP", "1024"))  # vocab columns per projection group
NPSB = int(os.environ.get("KNPSB", "4"))     # projection PSUM buffers
NGRP = (V + VGRP - 1) // VGRP


def ap3(base, offset, dims):
    """AP with the partition entry of ``base`` plus custom free dims."""
    return bass.AP(tensor=base.tensor, offset=base.offset + offset,
                   ap=[list(base.ap[0])] + [list(d) for d in dims])


def build_module(phases=("pre", "scan", "proj"),
                 fin_split=tuple(os.environ.get("KFIN", "v"))):
    nc = bacc.Bacc("TRN2", target_bir_lowering=False)
    tok_h = nc.dram_tensor("tok", (T,), I32, kind="ExternalInput")
    emb_h = nc.dram_tensor("embed", (V, E), F32, kind="ExternalInput")
    wih_h = nc.dram_tensor("wih", (E + 1, 2 * G3), F32, kind="ExternalInput")
    whhdd_h = nc.dram_tensor("whhdd", (H + 1, 2 * G3), F32, kind="ExternalInput")
    whhcn_h = nc.dram_tensor("whhcn", (H, 2 * G3), F32, kind="ExternalInput")
    wout_h = nc.dram_tensor("wout", (128, V), BF16, kind="ExternalInput")
    ones_h = nc.dram_tensor("onesrow", (1, 128), BF16, kind="ExternalInput")
    out_h = nc.dram_tensor("out", (T, V), BF16, kind="ExternalOutput")

    with tile.TileContext(nc) as tc:
        with ExitStack() as ctx:
            const = ctx.enter_context(tc.tile_pool(name="const", bufs=1))
            hall = ctx.enter_context(tc.tile_pool(name="hall", bufs=1))

            ident = const.tile([128, 128], F32, tag="ident")
            make_identity(nc, ident[:])
            wih_sb = const.tile([E + 1, 2 * G3], F32, tag="wih")
            nc.sync.dma_start(out=wih_sb[:], in_=wih_h[:])
            whhdd_sb = const.tile([H + 1, 2 * G3], F32, tag="whhdd")
            nc.sync.dma_start(out=whhdd_sb[:], in_=whhdd_h[:])
            whhcn_sb = const.tile([H, 2 * G3], F32, tag="whhcn")
            nc.sync.dma_start(out=whhcn_sb[:], in_=whhcn_h[:])
            tok_sb = const.tile([128, 4], I32, tag="tok")
            nc.sync.dma_start(out=tok_sb[:], in_=tok_h[:].rearrange("(g p) -> p g", p=128))

            woutc = hall.tile([128, V], BF16, tag="woutc")

            xt = const.tile([E + 1, T], F32, tag="xt")
            nc.vector.memset(xt[E:E + 1, :], 1.0)
            xtr = const.tile([E + 1, T], F32, tag="xtr")

            # Compact input-gate tensors, both direction blocks side by side.
            # Gate order is [z, r, n] so that tz lands at base partition 0
            # (BIR requires equal base partitions for two-SBUF-input ops).
            # gxc rows 0:64 = z/r gx (pad cols: z=+30, r=0); rows 64:96 = 0.
            # xnc = 2 * n-gate gx (pad cols 0).
            gxc = const.tile([G3, 2 * GXW], F32, tag="gxc")
            xnc = const.tile([H, 2 * GXW], F32, tag="xnc")
            for d in range(2):
                o = d * GXW
                nc.vector.memset(gxc[0:H, o:o + 2 * W], 30.0)
                nc.vector.memset(gxc[H:2 * H, o:o + 2 * W], 0.0)
                nc.vector.memset(xnc[:, o:o + 2 * W], 0.0)
            nc.vector.memset(gxc[2 * H:G3, :], 0.0)

            # h state by position, 32 partitions: cols [0, HBW) = L block,
            # [HBW, 2*HBW) = R block; real cols [2W, 2W+2S) within each block.
            hbufp = const.tile([H, 2 * HBW], F32, tag="hbufp")
            nc.vector.memset(hbufp[:], 0.0)

            # rings for zh' ([h;ones]), cn, cz, rz, n
            zhr, cnr, czr, rzr, nnr = [], [], [], [], []
            for j in range(3):
                zt = const.tile([H + 1, LAN], F32, tag=f"zh{j}", name=f"zh{j}")
                nc.vector.memset(zt[0:H, :], 0.0)
                nc.vector.memset(zt[H:H + 1, :], 1.0)
                zhr.append(zt)
                ct = const.tile([H, LAN], F32, tag=f"cn{j}", name=f"cn{j}")
                nc.vector.memset(ct[:], 0.0)
                cnr.append(ct)
                czt = const.tile([H, LAN], F32, tag=f"cz{j}", name=f"cz{j}")
                czr.append(czt)
                rzt = const.tile([2 * H, LAN], F32, tag=f"rz{j}", name=f"rz{j}")
                rzr.append(rzt)
                nnt = const.tile([H, LAN], F32, tag=f"nn{j}", name=f"nn{j}")
                nnr.append(nnt)
            uur = []
            for j in range(3):
                uut = const.tile([H, LAN], F32, tag=f"uu{j}", name=f"uu{j}")
                uur.append(uut)

            with (
                tc.tile_pool(name="gath", bufs=2) as gpool,
                tc.tile_pool(name="pps", bufs=2, space="PSUM") as ppre,
            ):
                if "pre" in phases:
                    # embedding gather + transpose to [E, tokens]
                    for g in range(4):
                        xg = gpool.tile([128, E], F32, tag="xg")
                        nc.gpsimd.indirect_dma_start(
                            out=xg[:], out_offset=None, in_=emb_h[:],
                            in_offset=bass.IndirectOffsetOnAxis(ap=tok_sb[:, g:g + 1], axis=0),
                        )
                        xps = ppre.tile([E, 128], F32, tag="ps")
                        nc.tensor.transpose(xps[:], xg[:], ident[:])
                        nc.scalar.copy(out=xt[0:E, g * 128:(g + 1) * 128], in_=xps[:])

                    # time-reversed copy (pairwise: s reversed, b kept)
                    src = xt[:]
                    nc.vector.tensor_copy(
                        out=xtr[:],
                        in_=ap3(src, 2 * (S - 1), [[-2, S], [1, 2]]),
                    )

                    # input-gate matmuls -> compact tiles
                    for d, rhs in ((0, xt), (1, xtr)):
                        o = d * GXW
                        prz = ppre.tile([2 * H, T], F32, tag="prz")
                        nc.tensor.matmul(prz[:], wih_sb[:, d * G3:d * G3 + 2 * H], rhs[:],
                                         start=True, stop=True)
                        nc.vector.tensor_copy(out=gxc[0:2 * H, o + 2 * W:o + 2 * W + T], in_=prz[:])
                        pn = ppre.tile([H, T], F32, tag="pn")
                        nc.tensor.matmul(pn[:], wih_sb[:, d * G3 + 2 * H:(d + 1) * G3], rhs[:],
                                         start=True, stop=True)
                        nc.vector.tensor_copy(out=xnc[:, o + 2 * W:o + 2 * W + T], in_=pn[:])

                # wout cache DMA, issued from the gpsimd queue AFTER the embed
                # gathers (the DMA-engine pool is a serial FIFO; this keeps the
                # big load from delaying the scan start).
                for c0 in range(0, V, 4096):
                    cw = min(4096, V - c0)
                    nc.gpsimd.dma_start(out=woutc[:, c0:c0 + cw], in_=wout_h[:][:, c0:c0 + cw])

            # ---- chunked fused scan ----
            with tc.tile_pool(name="scp", bufs=3, space="PSUM") as scp:
                for t in range(STEPS if "scan" in phases else 0):
                    pt = scp.tile([128, LAN], F32, tag="sp")
                    gh = pt[0:G3, 0:LAN]
                    uu = uur[t % 3][:]
                    zhp = zhr[(t - 1) % 3]
                    cnp = cnr[(t - 1) % 3]
                    zhc = zhr[t % 3]
                    cnc = cnr[t % 3]
                    czc = czr[t % 3]
                    rzv = rzr[t % 3][:]
                    nnv = nnr[t % 3][:]

                    # gates(t) = gx(t) + 0.5*Whh @ zh'(t-1) + Whh @ cn(t-1)
                    gsrc = ap3(gxc[0:G3, :], 2 * t, [[GXW, 2], [2 * P, C], [1, 2]])
                    nc.tensor.matmul(gh, ident[0:G3, 0:G3], gsrc,
                                     start=True, stop=False, skip_group_check=True)
                    for d in range(2):
                        nc.tensor.matmul(
                            gh[:, d * LPD:(d + 1) * LPD],
                            whhdd_sb[:, d * G3:(d + 1) * G3], zhp[:, d * LPD:(d + 1) * LPD],
                            start=False, stop=False, skip_group_check=True)
                    for d in range(2):
                        nc.tensor.matmul(
                            gh[:, d * LPD:(d + 1) * LPD],
                            whhcn_sb[:, d * G3:(d + 1) * G3], cnp[:, d * LPD:(d + 1) * LPD],
                            start=False, stop=(d == 1), skip_group_check=True)

                    # z,r = sigmoid = .5 + .5*tanh(x/2)  (gate order [z, r, n])
                    nc.scalar.activation(out=rzv, in_=pt[0:2 * H, 0:LAN], func=AF.Tanh, scale=0.5)

                    # n path first in the DVE stream (critical):
                    # u = (tr+1)*hn + 2*xn ; n = tanh(u/2)
                    nc.vector.scalar_tensor_tensor(
                        out=uu, in0=rzv[H:2 * H, :], scalar=1.0,
                        in1=pt[2 * H:G3, 0:LAN], op0=ALU.add, op1=ALU.mult)
                    usrc = ap3(xnc[0:H, :], 2 * t, [[GXW, 2], [2 * P, C], [1, 2]])
                    nc.vector.tensor_tensor(uu, uu, usrc, ALU.add)
                    nc.scalar.activation(out=nnv, in_=uu, func=AF.Tanh, scale=0.5)

                    # off-path: zh'(t) = (tz+1)*h(t-1) (skip at t=0: ring
                    # holds zeros), cz = (1-z) = .5 - .5*tz (Pool)
                    if t > 0:
                        hl = ap3(hbufp[0:H, :], 2 * (t - 1), [[2 * P, C], [1, 2]])
                        nc.vector.scalar_tensor_tensor(
                            out=zhc[0:H, 0:LPD], in0=rzv[0:H, 0:LPD],
                            scalar=1.0, in1=hl, op0=ALU.add, op1=ALU.mult)
                        hr = ap3(hbufp[0:H, :], HBW + (4 * W + 2 * S - 2) - 2 * (t - 1),
                                 [[-2 * P, C], [1, 2]])
                        nc.vector.scalar_tensor_tensor(
                            out=zhc[0:H, LPD:LAN], in0=rzv[0:H, LPD:LAN],
                            scalar=1.0, in1=hr, op0=ALU.add, op1=ALU.mult)
                    nc.gpsimd.tensor_scalar(czc[:], rzv[0:H, :], -0.5, 0.5,
                                            ALU.mult, ALU.add)

                    # cn = (1-z)*n ; h(t) = cn + 0.5*zh'
                    nc.vector.tensor_tensor(cnc[:], nnv, czc[:], ALU.mult)
                    hl = ap3(hbufp[0:H, :], 2 * t, [[2 * P, C], [1, 2]])
                    nc.vector.scalar_tensor_tensor(
                        out=hl, in0=zhc[0:H, 0:LPD], scalar=0.5, in1=cnc[:, 0:LPD],
                        op0=ALU.mult, op1=ALU.add)
                    hr = ap3(hbufp[0:H, :], HBW + (4 * W + 2 * S - 2) - 2 * t,
                             [[-2 * P, C], [1, 2]])
                    nc.vector.scalar_tensor_tensor(
                        out=hr, in0=zhc[0:H, LPD:LAN], scalar=0.5, in1=cnc[:, LPD:LAN],
                        op0=ALU.mult, op1=ALU.add)

            # ---- pack shells (bf16, 128 tokens):
            # rows [L-hi 0:32 | R-hi 32:64 | L-lo 64:96 | R-lo 96:127 | ones 127]
            hsh = []
            for k in range(4):
                hs = hall.tile([128, 128], BF16, tag=f"hs{k}", name=f"hs{k}")
                cl = 2 * W + 128 * k
                cr = HBW + 2 * W + 128 * k
                nc.vector.tensor_copy(out=hs[0:H, :], in_=hbufp[:, cl:cl + 128])
                nc.vector.tensor_copy(out=hs[H:2 * H, :], in_=hbufp[:, cr:cr + 128])
                nc.vector.tensor_tensor(hs[2 * H:G3, :], hbufp[:, cl:cl + 128],
                                        hs[0:H, :], ALU.subtract)
                hr16 = hall.tile([H, 128], BF16, tag=f"hr{k}", name=f"hr16_{k}")
                nc.vector.tensor_copy(out=hr16[:], in_=hbufp[:, cr:cr + 128])
                nc.vector.tensor_tensor(hr16[0:H - 1, :], hbufp[0:H - 1, cr:cr + 128],
                                        hr16[0:H - 1, :], ALU.subtract)
                nc.vector.tensor_copy(out=hs[G3:127, :], in_=hr16[0:H - 1, :])
                nc.sync.dma_start(out=hs[127:128, :], in_=ones_h[:])
                hsh.append(hs)

            do_proj = "proj" in phases
            if not do_proj and "scan" not in phases:
                for k in range(4):
                    nc.vector.memset(hsh[k][:], 0.0)

            # ---- projection + log-softmax ----
            opool = ctx.enter_context(tc.tile_pool(name="outp", bufs=4))
            with (
                tc.tile_pool(name="pp", bufs=NPSB, space="PSUM") as ppool,
                tc.tile_pool(name="esc", bufs=6) as epool,
            ):
                stats = [const.tile([128, NGRP], F32, tag=f"st{k}", name=f"stats{k}")
                         for k in range(4)]
                negc = [const.tile([128, 1], F32, tag=f"ng{k}", name=f"negc{k}")
                        for k in range(4)]

                def groups():
                    for g in range(NGRP):
                        c0 = g * VGRP
                        yield g, c0, min(VGRP, V - c0)

                def mms(k, g, c0, gw, tag):
                    ps = ppool.tile([128, VGRP], F32, tag="pp", name=f"pp{tag}{k}_{g}")
                    for q0 in range(0, gw, 512):
                        qw = min(512, gw - q0)
                        nc.tensor.matmul(ps[:, q0:q0 + qw], hsh[k][:],
                                         woutc[:, c0 + q0:c0 + q0 + qw],
                                         start=True, stop=True)
                    return ps

                def p1(k, g, c0, gw, pool_red=False):
                    ps = mms(k, g, c0, gw, "a")
                    esc = epool.tile([128, VGRP], BF16, tag="esc", name=f"esc{k}_{g}")
                    if pool_red:
                        # sum on Pool from the bf16 exp scratch; saves the
                        # ACT accumulator-read overhead on the critical engine
                        nc.scalar.activation(out=esc[:, 0:gw], in_=ps[:, 0:gw], func=AF.Exp)
                        nc.gpsimd.tensor_scalar(esc[:, 0:gw], esc[:, 0:gw], 1.0, 0.0,
                                                ALU.mult, ALU.add,
                                                accum_out=stats[k][:, g:g + 1])
                    else:
                        nc.scalar.activation(out=esc[:, 0:gw], in_=ps[:, 0:gw], func=AF.Exp,
                                             accum_out=stats[k][:, g:g + 1])

                def lse(k):
                    # negc = -ln(ssum) without the Ln table (exp stays loaded):
                    # crude log from the float bits, then one Newton step
                    # y1 = y0 - 1 + s*exp(-y0); |err| <= ~5e-4.
                    ssum = const.tile([128, 1], F32, tag=f"ss{k}", name=f"ssum{k}")
                    nc.vector.tensor_reduce(out=ssum[:], in_=stats[k][:],
                                            axis=mybir.AxisListType.X, op=ALU.add)
                    y0 = const.tile([128, 1], F32, tag=f"y0{k}", name=f"y0_{k}")
                    nc.vector.tensor_copy(out=y0[:], in_=ssum[:].bitcast(I32))
                    nc.vector.tensor_scalar(y0[:], y0[:], 8.2629582e-8, -87.999887,
                                            ALU.mult, ALU.add)
                    ex = const.tile([128, 1], F32, tag=f"ex{k}", name=f"ex_{k}")
                    nc.scalar.activation(out=ex[:], in_=y0[:], func=AF.Exp, scale=-1.0)
                    nc.vector.tensor_tensor(ex[:], ex[:], ssum[:], ALU.mult)
                    nc.vector.tensor_tensor(ex[:], ex[:], y0[:], ALU.add)
                    nc.vector.tensor_scalar(negc[k][:], ex[:], -1.0, 1.0,
                                            ALU.mult, ALU.add)

                def p2(k, g, c0, gw, split=None):
                    ps = mms(k, g, c0, gw, "b")
                    ob = opool.tile([128, VGRP], BF16, tag="ob", name=f"ob{k}_{g}")
                    if split == "half":
                        # drain mode: ACT and DVE each finalize half the tile
                        h0 = (gw + 1) // 2
                        nc.scalar.activation(out=ob[:, 0:h0], in_=ps[:, 0:h0],
                                             func=AF.Identity, bias=negc[k][:, 0:1])
                        nc.vector.tensor_scalar_add(ob[:, h0:gw], ps[:, h0:gw],
                                                    negc[k][:, 0:1])
                    else:
                        split = fin_split if split is None else split
                        eng = split[g % len(split)]
                        if eng == "v":
                            nc.vector.tensor_scalar_add(ob[:, 0:gw], ps[:, 0:gw],
                                                        negc[k][:, 0:1])
                        else:
                            nc.scalar.activation(out=ob[:, 0:gw], in_=ps[:, 0:gw],
                                                 func=AF.Identity, bias=negc[k][:, 0:1])
                    dst = bass.AP(tensor=out_h[:].tensor, offset=(128 * k) * V + c0,
                                  ap=[[V, 128], [1, gw]])
                    nc.sync.dma_start(out=dst, in_=ob[:, 0:gw])

                if do_proj:
                    for g, c0, gw in groups():
                        p1(0, g, c0, gw)
                    for k in range(4):
                        lse(k)
                        if k < 3:
                            for g, c0, gw in groups():
                                p1(k + 1, g, c0, gw)
                                p2(k, g, c0, gw)
                        else:
                            # drain: no pass-1 left; ACT and DVE each take
                            # half of every finalize tile
                            for g, c0, gw in groups():
                                p2(k, g, c0, gw, split="half")
    nc.compile()
    return nc


_CACHE = {}


def _get_module():
    if "nc" not in _CACHE:
        _CACHE["nc"] = build_module()
    return _CACHE["nc"]


def prep_inputs(inputs):
    """Host-side prep: build per-core input maps from the full input dict."""
    import ml_dtypes

    ib = np.asarray(inputs["input_batch"])
    embed = np.ascontiguousarray(np.asarray(inputs["embed"], dtype=np.float32))

    def f32(x):
        return np.asarray(x, dtype=np.float32)

    # Gate order [z, r, n] (see kernel); n-gate input weights pre-doubled.
    perm = np.concatenate([np.arange(H, 2 * H), np.arange(0, H),
                           np.arange(2 * H, G3)])
    nsc = np.concatenate([np.ones(2 * H, np.float32), 2.0 * np.ones(H, np.float32)])

    wih = np.zeros((E + 1, 2 * G3), np.float32)
    for d, (Wd, bd) in enumerate(((inputs["Wl_ih"], inputs["bl_ih"]),
                                  (inputs["Wr_ih"], inputs["br_ih"]))):
        Wd, bd = f32(Wd), f32(bd)
        o = d * G3
        wih[:E, o:o + G3] = Wd[:, perm] * nsc
        wih[E, o:o + G3] = bd[perm] * nsc

    whhdd = np.zeros((H + 1, 2 * G3), np.float32)
    whhcn = np.zeros((H, 2 * G3), np.float32)
    for d, (Wd, bd) in enumerate(((inputs["Wl_hh"], inputs["bl_hh"]),
                                  (inputs["Wr_hh"], inputs["br_hh"]))):
        Wd, bd = f32(Wd), f32(bd)
        o = d * G3
        whhdd[:H, o:o + G3] = 0.5 * Wd[:, perm]
        whhdd[H, o:o + G3] = bd[perm]
        whhcn[:, o:o + G3] = Wd[:, perm]

    rnn_out = f32(inputs["rnn_out"])
    rnn_out_bias = f32(inputs["rnn_out_bias"])
    wout = np.zeros((128, V), np.float32)
    wout[0:2 * H] = rnn_out                  # hi rows (L then R)
    wout[2 * H:127] = rnn_out[0:2 * H - 1]   # lo rows (L 32, R 31)
    wout[127] = rnn_out_bias[0]              # ones row
    woutp = wout.astype(ml_dtypes.bfloat16)

    in_maps = []
    for c in range(NCORES):
        tok = np.ascontiguousarray(
            ib[:, BC * c:BC * (c + 1)].astype(np.int32).reshape(T)
        )
        in_maps.append(
            {"tok": tok, "embed": embed, "wih": wih, "whhdd": whhdd,
             "whhcn": whhcn, "wout": woutp,
             "onesrow": np.ones((1, 128), ml_dtypes.bfloat16)}
        )
    return in_maps


def assemble_output(results):
    out = np.empty((S, B, V), np.float32)
    for c in range(NCORES):
        out[:, BC * c:BC * (c + 1), :] = (
            results[c]["out"].astype(np.float32).reshape(S, BC, V)
        )
    return out


def kernel(**inputs):
    from concourse.bass_utils import run_bass_kernel_spmd

    nc = _get_module()
    in_maps = prep_inputs(inputs)
    res = run_bass_kernel_spmd(nc, in_maps, core_ids=list(range(NCORES)))
    return assemble_output(res.results)


# revision 49
# speedup vs baseline: 3.0002x; 1.0395x over previous
"""BiRNN (bidirectional GRU) language model kernel for Trainium2, 8 NeuronCores.

Sharding: data-parallel over batch (2 of 16 batch columns per core), no
collectives.  Each core computes the embedding gather, both GRU scans, the
vocab projection and the log-softmax for its 512 tokens.

Key structure:
  - Chunked scan: each direction's 256-step recurrence is split into C chunks
    run in parallel as lanes of the same instructions, each with a W-step
    warmup ramp.  Positions before the sequence start are padded with a
    frozen-gate column (z-gate pre-activation +30 => z=1 => h stays exactly 0),
    so chunks whose warmup crosses position 0 are exact and later chunks
    converge through the GRU's state contraction (validated: W=32 gives
    ~5e-7 max h error).  Serial steps: 256 -> 256/C + W.
  - Per step, the gate matmul Whh @ h(t-1) is split into an accumulation of
    three PSUM matmuls: identity @ gx(t) (the precomputed input gates),
    0.5*Whh @ zh'(t-1) and Whh @ cn(t-1), where h = cn + 0.5*zh'
    (cn = (1-z)*n, zh' = 2z*h).  This keeps the recurrence's serial path at
    mm -> tanh(rz) -> (r+1)*hn -> +xn -> tanh -> *(1-z) -> mm.
  - h states land in a position-indexed buffer (later chunks' warmup writes
    are overwritten by the owning chunk's real writes, which come later in
    step order).
  - Vocab projection: h split bf16 hi/lo packed into one K=128 stationary
    tile per 128 tokens ([h-hi 64 | ones | h-lo 63]); wout packed to match
    ([w-hi 64 | bias | w-hi 63]) so one bf16 matmul per 512 columns gives
    logits to ~2e-3 abs.  wout ([128, V] bf16, 12.9MB) is fully cached in
    SBUF, loaded during the scan.
  - log-softmax without a max pass (|logits| <= 65 so exp can't overflow):
    pass 1 Exp+accum_out per 2048-col group; pass 2 recomputes logits and
    writes logits - log(sum) as bf16 (host converts to f32).  Pass 2 of
    shell k is interleaved with pass 1 of shell k+1; the finalize add is
    split across DVE and Pool so ACT only does the Exps.
"""

import os
import sys
from contextlib import ExitStack

import numpy as np

for _p in (
    "/opt/trn_rl_repo",
    "/root/.axon_site",
    "/root/.axon_site/_ro/trn_rl_repo",
    "/root/.axon_site/_ro/pypackages",
):
    if os.path.isdir(_p) and _p not in sys.path:
        sys.path.append(_p)

import concourse.bass as bass
import concourse.bacc as bacc
import concourse.tile as tile
from concourse import mybir
from concourse.masks import make_identity

F32 = mybir.dt.float32
BF16 = mybir.dt.bfloat16
I32 = mybir.dt.int32
AF = mybir.ActivationFunctionType
ALU = mybir.AluOpType

V = 50257
E = 64
H = 32
S = 256
B = 16
NCORES = 8
BC = B // NCORES          # batch columns per core
T = S * BC                # tokens per core
G3 = 3 * H                # 96 gate rows (r, z, n)

C = int(os.environ.get("KC", "32"))    # chunks per direction
W = int(os.environ.get("KW", "24"))    # warmup steps
P = S // C                              # positions per chunk
STEPS = P + W
LPD = 2 * C                             # lanes per direction (chunk x batchcol)
LAN = 2 * LPD                           # total lanes (L block, R block)
GXW = 2 * W + 2 * S                     # compact gx width per direction block
HBW = 2 * S + 4 * W                     # h buffer width (L pad left, R pad right)

VGRP = int(os.environ.get("KVGRP", "1024"))  # vocab columns per projection group
NPSB = int(os.environ.get("KNPSB", "4"))     # projection PSUM buffers
NGRP = (V + VGRP - 1) // VGRP


def ap3(base, offset, dims):
    """AP with the partition entry of ``base`` plus custom free dims."""
    return bass.AP(tensor=base.tensor, offset=base.offset + offset,
                   ap=[list(base.ap[0])] + [list(d) for d in dims])


def build_module(phases=("pre", "scan", "proj"),
                 fin_split=tuple(os.environ.get("KFIN", "v"))):
    nc = bacc.Bacc("TRN2", target_bir_lowering=False)
    tok_h = nc.dram_tensor("tok", (T,), I32, kind="ExternalInput")
    emb_h = nc.dram_tensor("embed", (V, E), F32, kind="ExternalInput")
    wih_h = nc.dram_tensor("wih", (E + 1, 2 * G3), F32, kind="ExternalInput")
    whhdd_h = nc.dram_tensor("whhdd", (H + 1, 2 * G3), F32, kind="ExternalInput")
    whhcn_h = nc.dram_tensor("whhcn", (H, 2 * G3), F32, kind="ExternalInput")
    wout_h = nc.dram_tensor("wout", (128, V), BF16, kind="ExternalInput")
    ones_h = nc.dram_tensor("onesrow", (1, 128), BF16, kind="ExternalInput")
    out_h = nc.dram_tensor("out", (T, V), BF16, kind="ExternalOutput")

    with tile.TileContext(nc) as tc:
        with ExitStack() as ctx:
            const = ctx.enter_context(tc.tile_pool(name="const", bufs=1))
            hall = ctx.enter_context(tc.tile_pool(name="hall", bufs=1))

            ident = const.tile([128, 128], F32, tag="ident")
            make_identity(nc, ident[:])
            wih_sb = const.tile([E + 1, 2 * G3], F32, tag="wih")
            nc.sync.dma_start(out=wih_sb[:], in_=wih_h[:])
            whhdd_sb = const.tile([H + 1, 2 * G3], F32, tag="whhdd")
            nc.sync.dma_start(out=whhdd_sb[:], in_=whhdd_h[:])
            whhcn_sb = const.tile([H, 2 * G3], F32, tag="whhcn")
            nc.sync.dma_start(out=whhcn_sb[:], in_=whhcn_h[:])
            tok_sb = const.tile([128, 4], I32, tag="tok")
            nc.sync.dma_start(out=tok_sb[:], in_=tok_h[:].rearrange("(g p) -> p g", p=128))

            woutc = hall.tile([128, V], BF16, tag="woutc")

            xt = const.tile([E + 1, T], F32, tag="xt")
            nc.vector.memset(xt[E:E + 1, :], 1.0)
            xtr = const.tile([E + 1, T], F32, tag="xtr")

            # Compact input-gate tensors, both direction blocks side by side.
            # Gate order is [z, r, n] so that tz lands at base partition 0
            # (BIR requires equal base partitions for two-SBUF-input ops).
            # gxc rows 0:64 = z/r gx (pad cols: z=+30, r=0); rows 64:96 = 0.
            # xnc = 2 * n-gate gx (pad cols 0).
            gxc = const.tile([G3, 2 * GXW], F32, tag="gxc")
            xnc = const.tile([H, 2 * GXW], F32, tag="xnc")
            for d in range(2):
                o = d * GXW
                nc.vector.memset(gxc[0:H, o:o + 2 * W], 30.0)
                nc.vector.memset(gxc[H:2 * H, o:o + 2 * W], 0.0)
                nc.vector.memset(xnc[:, o:o + 2 * W], 0.0)
            nc.vector.memset(gxc[2 * H:G3, :], 0.0)

            # h state by position, 32 partitions: cols [0, HBW) = L block,
            # [HBW, 2*HBW) = R block; real cols [2W, 2W+2S) within each block.
            hbufp = const.tile([H, 2 * HBW], F32, tag="hbufp")
            nc.vector.memset(hbufp[:], 0.0)

            # rings for zh' ([h;ones]), cn, cz, rz, n
            zhr, cnr, czr, rzr, nnr = [], [], [], [], []
            for j in range(3):
                zt = const.tile([H + 1, LAN], F32, tag=f"zh{j}", name=f"zh{j}")
                nc.vector.memset(zt[0:H, :], 0.0)
                nc.vector.memset(zt[H:H + 1, :], 1.0)
                zhr.append(zt)
                ct = const.tile([H, LAN], F32, tag=f"cn{j}", name=f"cn{j}")
                nc.vector.memset(ct[:], 0.0)
                cnr.append(ct)
                czt = const.tile([H, LAN], F32, tag=f"cz{j}", name=f"cz{j}")
                czr.append(czt)
                rzt = const.tile([2 * H, LAN], F32, tag=f"rz{j}", name=f"rz{j}")
                rzr.append(rzt)
                nnt = const.tile([H, LAN], F32, tag=f"nn{j}", name=f"nn{j}")
                nnr.append(nnt)
            uur = []
            for j in range(3):
                uut = const.tile([H, LAN], F32, tag=f"uu{j}", name=f"uu{j}")
                uur.append(uut)

            with (
                tc.tile_pool(name="gath", bufs=2) as gpool,
                tc.tile_pool(name="pps", bufs=2, space="PSUM") as ppre,
            ):
                if "pre" in phases:
                    # embedding gather + transpose to [E, tokens]
                    for g in range(4):
                        xg = gpool.tile([128, E], F32, tag="xg")
                        nc.gpsimd.indirect_dma_start(
                            out=xg[:], out_offset=None, in_=emb_h[:],
                            in_offset=bass.IndirectOffsetOnAxis(ap=tok_sb[:, g:g + 1], axis=0),
                        )
                        xps = ppre.tile([E, 128], F32, tag="ps")
                        nc.tensor.transpose(xps[:], xg[:], ident[:])
                        nc.scalar.copy(out=xt[0:E, g * 128:(g + 1) * 128], in_=xps[:])

                    # time-reversed copy (pairwise: s reversed, b kept)
                    src = xt[:]
                    nc.vector.tensor_copy(
                        out=xtr[:],
                        in_=ap3(src, 2 * (S - 1), [[-2, S], [1, 2]]),
                    )

                    # input-gate matmuls -> compact tiles
                    for d, rhs in ((0, xt), (1, xtr)):
                        o = d * GXW
                        prz = ppre.tile([2 * H, T], F32, tag="prz")
                        nc.tensor.matmul(prz[:], wih_sb[:, d * G3:d * G3 + 2 * H], rhs[:],
                                         start=True, stop=True)
                        nc.vector.tensor_copy(out=gxc[0:2 * H, o + 2 * W:o + 2 * W + T], in_=prz[:])
                        pn = ppre.tile([H, T], F32, tag="pn")
                        nc.tensor.matmul(pn[:], wih_sb[:, d * G3 + 2 * H:(d + 1) * G3], rhs[:],
                                         start=True, stop=True)
                        nc.vector.tensor_copy(out=xnc[:, o + 2 * W:o + 2 * W + T], in_=pn[:])

                # wout cache DMA, issued from the gpsimd queue AFTER the embed
                # gathers (the DMA-engine pool is a serial FIFO; this keeps the
                # big load from delaying the scan start).
                for c0 in range(0, V, 4096):
                    cw = min(4096, V - c0)
                    nc.gpsimd.dma_start(out=woutc[:, c0:c0 + cw], in_=wout_h[:][:, c0:c0 + cw])

            # ---- chunked fused scan ----
            with tc.tile_pool(name="scp", bufs=3, space="PSUM") as scp:
                for t in range(STEPS if "scan" in phases else 0):
                    pt = scp.tile([128, LAN], F32, tag="sp")
                    gh = pt[0:G3, 0:LAN]
                    uu = uur[t % 3][:]
                    zhp = zhr[(t - 1) % 3]
                    cnp = cnr[(t - 1) % 3]
                    zhc = zhr[t % 3]
                    cnc = cnr[t % 3]
                    czc = czr[t % 3]
                    rzv = rzr[t % 3][:]
                    nnv = nnr[t % 3][:]

                    # gates(t) = gx(t) + 0.5*Whh @ zh'(t-1) + Whh @ cn(t-1)
                    gsrc = ap3(gxc[0:G3, :], 2 * t, [[GXW, 2], [2 * P, C], [1, 2]])
                    nc.tensor.matmul(gh, ident[0:G3, 0:G3], gsrc,
                                     start=True, stop=False, skip_group_check=True)
                    for d in range(2):
                        nc.tensor.matmul(
                            gh[:, d * LPD:(d + 1) * LPD],
                            whhdd_sb[:, d * G3:(d + 1) * G3], zhp[:, d * LPD:(d + 1) * LPD],
                            start=False, stop=False, skip_group_check=True)
                    for d in range(2):
                        nc.tensor.matmul(
                            gh[:, d * LPD:(d + 1) * LPD],
                            whhcn_sb[:, d * G3:(d + 1) * G3], cnp[:, d * LPD:(d + 1) * LPD],
                            start=False, stop=(d == 1), skip_group_check=True)

                    # z,r = sigmoid = .5 + .5*tanh(x/2)  (gate order [z, r, n])
                    nc.scalar.activation(out=rzv, in_=pt[0:2 * H, 0:LAN], func=AF.Tanh, scale=0.5)

                    # n path first in the DVE stream (critical):
                    # u = (tr+1)*hn + 2*xn ; n = tanh(u/2)
                    nc.vector.scalar_tensor_tensor(
                        out=uu, in0=rzv[H:2 * H, :], scalar=1.0,
                        in1=pt[2 * H:G3, 0:LAN], op0=ALU.add, op1=ALU.mult)
                    usrc = ap3(xnc[0:H, :], 2 * t, [[GXW, 2], [2 * P, C], [1, 2]])
                    nc.vector.tensor_tensor(uu, uu, usrc, ALU.add)
                    nc.scalar.activation(out=nnv, in_=uu, func=AF.Tanh, scale=0.5)

                    # off-path: zh'(t) = (tz+1)*h(t-1) (skip at t=0: ring
                    # holds zeros), cz = (1-z) = .5 - .5*tz (Pool)
                    if t > 0:
                        hl = ap3(hbufp[0:H, :], 2 * (t - 1), [[2 * P, C], [1, 2]])
                        nc.vector.scalar_tensor_tensor(
                            out=zhc[0:H, 0:LPD], in0=rzv[0:H, 0:LPD],
                            scalar=1.0, in1=hl, op0=ALU.add, op1=ALU.mult)
                        hr = ap3(hbufp[0:H, :], HBW + (4 * W + 2 * S - 2) - 2 * (t - 1),
                                 [[-2 * P, C], [1, 2]])
                        nc.vector.scalar_tensor_tensor(
                            out=zhc[0:H, LPD:LAN], in0=rzv[0:H, LPD:LAN],
                            scalar=1.0, in1=hr, op0=ALU.add, op1=ALU.mult)
                    nc.gpsimd.tensor_scalar(czc[:], rzv[0:H, :], -0.5, 0.5,
                                            ALU.mult, ALU.add)

                    # cn = (1-z)*n ; h(t) = cn + 0.5*zh'
                    nc.vector.tensor_tensor(cnc[:], nnv, czc[:], ALU.mult)
                    hl = ap3(hbufp[0:H, :], 2 * t, [[2 * P, C], [1, 2]])
                    nc.vector.scalar_tensor_tensor(
                        out=hl, in0=zhc[0:H, 0:LPD], scalar=0.5, in1=cnc[:, 0:LPD],
                        op0=ALU.mult, op1=ALU.add)
                    hr = ap3(hbufp[0:H, :], HBW + (4 * W + 2 * S - 2) - 2 * t,
                             [[-2 * P, C], [1, 2]])
                    nc.vector.scalar_tensor_tensor(
                        out=hr, in0=zhc[0:H, LPD:LAN], scalar=0.5, in1=cnc[:, LPD:LAN],
                        op0=ALU.mult, op1=ALU.add)

            # ---- pack shells (bf16, 128 tokens):
            # rows [L-hi 0:32 | R-hi 32:64 | L-lo 64:96 | R-lo 96:127 | ones 127]
            hsh = []
            for k in range(4):
                hs = hall.tile([128, 128], BF16, tag=f"hs{k}", name=f"hs{k}")
                cl = 2 * W + 128 * k
                cr = HBW + 2 * W + 128 * k
                nc.vector.tensor_copy(out=hs[0:H, :], in_=hbufp[:, cl:cl + 128])
                nc.vector.tensor_copy(out=hs[H:2 * H, :], in_=hbufp[:, cr:cr + 128])
                nc.vector.tensor_tensor(hs[2 * H:G3, :], hbufp[:, cl:cl + 128],
                                        hs[0:H, :], ALU.subtract)
                hr16 = hall.tile([H, 128], BF16, tag=f"hr{k}", name=f"hr16_{k}")
                nc.vector.tensor_copy(out=hr16[:], in_=hbufp[:, cr:cr + 128])
                nc.vector.tensor_tensor(hr16[0:H - 1, :], hbufp[0:H - 1, cr:cr + 128],
                                        hr16[0:H - 1, :], ALU.subtract)
                nc.vector.tensor_copy(out=hs[G3:127, :], in_=hr16[0:H - 1, :])
                nc.sync.dma_start(out=hs[127:128, :], in_=ones_h[:])
                hsh.append(hs)

            do_proj = "proj" in phases
            if not do_proj and "scan" not in phases:
                for k in range(4):
                    nc.vector.memset(hsh[k][:], 0.0)

            # ---- projection + log-softmax ----
            opool = ctx.enter_context(tc.tile_pool(name="outp", bufs=4))
            with (
                tc.tile_pool(name="pp", bufs=NPSB, space="PSUM") as ppool,
                tc.tile_pool(name="esc", bufs=6) as epool,
            ):
                stats = [const.tile([128, NGRP], F32, tag=f"st{k}", name=f"stats{k}")
                         for k in range(4)]
                negc = [const.tile([128, 1], F32, tag=f"ng{k}", name=f"negc{k}")
                        for k in range(4)]

                def groups():
                    for g in range(NGRP):
                        c0 = g * VGRP
                        yield g, c0, min(VGRP, V - c0)

                def mms(k, g, c0, gw, tag):
                    ps = ppool.tile([128, VGRP], F32, tag="pp", name=f"pp{tag}{k}_{g}")
                    for q0 in range(0, gw, 512):
                        qw = min(512, gw - q0)
                        nc.tensor.matmul(ps[:, q0:q0 + qw], hsh[k][:],
                                         woutc[:, c0 + q0:c0 + q0 + qw],
                                         start=True, stop=True)
                    return ps

                def p1(k, g, c0, gw, dve_red=False):
                    ps = mms(k, g, c0, gw, "a")
                    esc = epool.tile([128, VGRP], BF16, tag="esc", name=f"esc{k}_{g}")
                    if dve_red:
                        # sum on DVE (2x mode on the bf16 scratch); saves the
                        # ACT accumulator-read overhead on the critical engine
                        nc.scalar.activation(out=esc[:, 0:gw], in_=ps[:, 0:gw], func=AF.Exp)
                        nc.vector.tensor_reduce(out=stats[k][:, g:g + 1], in_=esc[:, 0:gw],
                                                axis=mybir.AxisListType.X, op=ALU.add)
                    else:
                        nc.scalar.activation(out=esc[:, 0:gw], in_=ps[:, 0:gw], func=AF.Exp,
                                             accum_out=stats[k][:, g:g + 1])

                def lse(k):
                    # negc = -ln(ssum) without the Ln table (exp stays loaded):
                    # crude log from the float bits, then one Newton step
                    # y1 = y0 - 1 + s*exp(-y0); |err| <= ~5e-4.
                    ssum = const.tile([128, 1], F32, tag=f"ss{k}", name=f"ssum{k}")
                    nc.vector.tensor_reduce(out=ssum[:], in_=stats[k][:],
                                            axis=mybir.AxisListType.X, op=ALU.add)
                    y0 = const.tile([128, 1], F32, tag=f"y0{k}", name=f"y0_{k}")
                    nc.vector.tensor_copy(out=y0[:], in_=ssum[:].bitcast(I32))
                    nc.vector.tensor_scalar(y0[:], y0[:], 8.2629582e-8, -87.999887,
                                            ALU.mult, ALU.add)
                    ex = const.tile([128, 1], F32, tag=f"ex{k}", name=f"ex_{k}")
                    nc.scalar.activation(out=ex[:], in_=y0[:], func=AF.Exp, scale=-1.0)
                    nc.vector.tensor_tensor(ex[:], ex[:], ssum[:], ALU.mult)
                    nc.vector.tensor_tensor(ex[:], ex[:], y0[:], ALU.add)
                    nc.vector.tensor_scalar(negc[k][:], ex[:], -1.0, 1.0,
                                            ALU.mult, ALU.add)

                def p2(k, g, c0, gw, split=None):
                    ps = mms(k, g, c0, gw, "b")
                    ob = opool.tile([128, VGRP], BF16, tag="ob", name=f"ob{k}_{g}")
                    if split == "half":
                        # drain mode: ACT and DVE each finalize half the tile
                        h0 = (gw + 1) // 2
                        nc.scalar.activation(out=ob[:, 0:h0], in_=ps[:, 0:h0],
                                             func=AF.Identity, bias=negc[k][:, 0:1])
                        nc.vector.tensor_scalar_add(ob[:, h0:gw], ps[:, h0:gw],
                                                    negc[k][:, 0:1])
                    else:
                        split = fin_split if split is None else split
                        eng = split[g % len(split)]
                        if eng == "v":
                            nc.vector.tensor_scalar_add(ob[:, 0:gw], ps[:, 0:gw],
                                                        negc[k][:, 0:1])
                        else:
                            nc.scalar.activation(out=ob[:, 0:gw], in_=ps[:, 0:gw],
                                                 func=AF.Identity, bias=negc[k][:, 0:1])
                    dst = bass.AP(tensor=out_h[:].tensor, offset=(128 * k) * V + c0,
                                  ap=[[V, 128], [1, gw]])
                    nc.sync.dma_start(out=dst, in_=ob[:, 0:gw])

                if do_proj:
                    for g, c0, gw in groups():
                        p1(0, g, c0, gw, dve_red=True)
                    for k in range(4):
                        lse(k)
                        if k < 3:
                            for g, c0, gw in groups():
                                p1(k + 1, g, c0, gw)
                                p2(k, g, c0, gw)
                        else:
                            # drain: no pass-1 left; ACT and DVE each take
                            # half of every finalize tile
                            for g, c0, gw in groups():
                                p2(k, g, c0, gw, split="half")
    nc.compile()
    return nc


_CACHE = {}


def _get_module():
    if "nc" not in _CACHE:
        _CACHE["nc"] = build_module()
    return _CACHE["nc"]


def prep_inputs(inputs):
    """Host-side prep: build per-core input maps from the full input dict."""
    import ml_dtypes

    ib = np.asarray(inputs["input_batch"])
    embed = np.ascontiguousarray(np.asarray(inputs["embed"], dtype=np.float32))

    def f32(x):
        return np.asarray(x, dtype=np.float32)

    # Gate order [z, r, n] (see kernel); n-gate input weights pre-doubled.
    perm = np.concatenate([np.arange(H, 2 * H), np.arange(0, H),
                           np.arange(2 * H, G3)])
    nsc = np.concatenate([np.ones(2 * H, np.float32), 2.0 * np.ones(H, np.float32)])

    wih = np.zeros((E + 1, 2 * G3), np.float32)
    for d, (Wd, bd) in enumerate(((inputs["Wl_ih"], inputs["bl_ih"]),
                                  (inputs["Wr_ih"], inputs["br_ih"]))):
        Wd, bd = f32(Wd), f32(bd)
        o = d * G3
        wih[:E, o:o + G3] = Wd[:, perm] * nsc
        wih[E, o:o + G3] = bd[perm] * nsc

    whhdd = np.zeros((H + 1, 2 * G3), np.float32)
    whhcn = np.zeros((H, 2 * G3), np.float32)
    for d, (Wd, bd) in enumerate(((inputs["Wl_hh"], inputs["bl_hh"]),
                                  (inputs["Wr_hh"], inputs["br_hh"]))):
        Wd, bd = f32(Wd), f32(bd)
        o = d * G3
        whhdd[:H, o:o + G3] = 0.5 * Wd[:, perm]
        whhdd[H, o:o + G3] = bd[perm]
        whhcn[:, o:o + G3] = Wd[:, perm]

    rnn_out = f32(inputs["rnn_out"])
    rnn_out_bias = f32(inputs["rnn_out_bias"])
    wout = np.zeros((128, V), np.float32)
    wout[0:2 * H] = rnn_out                  # hi rows (L then R)
    wout[2 * H:127] = rnn_out[0:2 * H - 1]   # lo rows (L 32, R 31)
    wout[127] = rnn_out_bias[0]              # ones row
    woutp = wout.astype(ml_dtypes.bfloat16)

    in_maps = []
    for c in range(NCORES):
        tok = np.ascontiguousarray(
            ib[:, BC * c:BC * (c + 1)].astype(np.int32).reshape(T)
        )
        in_maps.append(
            {"tok": tok, "embed": embed, "wih": wih, "whhdd": whhdd,
             "whhcn": whhcn, "wout": woutp,
             "onesrow": np.ones((1, 128), ml_dtypes.bfloat16)}
        )
    return in_maps


def assemble_output(results):
    out = np.empty((S, B, V), np.float32)
    for c in range(NCORES):
        out[:, BC * c:BC * (c + 1), :] = (
            results[c]["out"].astype(np.float32).reshape(S, BC, V)
        )
    return out


def kernel(**inputs):
    from concourse.bass_utils import run_bass_kernel_spmd

    nc = _get_module()
    in_maps = prep_inputs(inputs)
    res = run_bass_kernel_spmd(nc, in_maps, core_ids=list(range(NCORES)))
    return assemble_output(res.results)
